# revision 1
# baseline (speedup 1.0000x reference)
"""Trainium2 Bass kernel for nn_CompositionalMLP_75763223101514.

Reference computation (per batch row b, expert k):
    xb = x.reshape(B, 16, 128)
    h  = leaky( einsum('bkm,kdm->bkd', xb, W1diag) + b1 )    # W1diag[k] = W1[k,:,k*128:(k+1)*128]
    o  = leaky( einsum('bkd,kld->bkl', h, W2) + b2 )
    out = o.reshape(B, 16*128)
with leaky(z) = z if z > 0 else 0.2 z.

Strategy: data-parallel over the batch dim across 8 NeuronCores (2048 rows
each), weights replicated.  On the host we pre-transpose each x shard to
feature-major [2048, 2048] so the contraction dim (m) lands on SBUF
partitions, extract the diagonal W1 blocks, and pre-transpose the weights
into lhsT layout.  Each core then runs, per expert k:

    MM1:  hT[d, b]  = sum_m W1T_k[m, d] * xT[k*128+m, b]     (PE, contraction 128)
    act:  h = leaky(hT + b1)  (ScalarE Prelu for the d<128 chunk + the output;
                               VectorE 2-op max(z, 0.2z) for the d>=128 chunk)
    MM2:  oT[l, b]  = sum_d W2T_k[d, l] * h[d, b]            (PE, contraction 240, accumulated)
    act:  o = leaky(oT + b2)  -> SBUF -> DMA to oT dram [k*128+l, b]

The host finally re-transposes each core's oT shard back to [2048, 2048]
batch-major and concatenates.

Matmul dtype: float32r (single-pass fp32 on the PE at full bf16 rate for
moving dim >= 256; measured max rel err ~1.4e-4 per matmul vs 2.5e-3 for
bf16).  Set DT = "bf16" to halve input DMA instead.
"""

import numpy as np
import ml_dtypes

import concourse.bacc as bacc
import concourse.mybir as mybir
from concourse.tile import TileContext
from concourse.bass_utils import run_bass_kernel_spmd

K, M, DK, L = 16, 128, 240, 128
B = 16384
NCORES = 8
BL = B // NCORES          # batch rows per core
SLOPE = 0.2
DA, DB = 128, DK - 128    # hidden split (PSUM partition limit)

DT = "fp32r"              # "bf16" | "fp32r" | "fp32"
BT = 1024                 # activation tile width (columns of local batch)

_DTYPES = {
    "bf16": (mybir.dt.bfloat16, ml_dtypes.bfloat16),
    "fp32r": (mybir.dt.float32r, np.float32),
    "fp32": (mybir.dt.float32, np.float32),
}

A = mybir.ActivationFunctionType
OP = mybir.AluOpType

_cache = {}


def _build(dt_name, repeat=1):
    """One SPMD program; all cores run it on their own batch shard."""
    dt_mm, _ = _DTYPES[dt_name]
    f32 = mybir.dt.float32
    nc = bacc.Bacc("TRN2", target_bir_lowering=False, debug=False, num_devices=NCORES)

    xT = nc.dram_tensor("xT", [K * M, BL], dt_mm, kind="ExternalInput")
    w1t = nc.dram_tensor("w1t", [K, M, DK], dt_mm, kind="ExternalInput")   # [k][m, d]
    w2t = nc.dram_tensor("w2t", [K, DK, L], dt_mm, kind="ExternalInput")   # [k][d, l]
    # bias pack: [:, k, 0]=b1[:128]  [:112, k, 1]=b1[128:]  [:, k, 2]=0.2*b1[:128]
    #            [:112, k, 3]=0.2*b1[128:]  [:, k, 4]=b2  [:, k, 5]=0.2*b2
    bias = nc.dram_tensor("bias", [128, K, 6], f32, kind="ExternalInput")
    oT = nc.dram_tensor("oT", [K * L, BL], f32, kind="ExternalOutput")

    n_half = BL // BT           # halves per expert
    n_mm = BT // NMM            # matmuls per half per chunk

    with TileContext(nc) as tc:
        with (
            tc.tile_pool(name="const", bufs=1) as cpool,
            tc.tile_pool(name="xin", bufs=XBUFS) as xpool,
            tc.tile_pool(name="h", bufs=2) as hpool,
            tc.tile_pool(name="o", bufs=OBUFS) as opool,
            tc.tile_pool(name="psum", bufs=1, space="PSUM") as psum,
        ):
            # --- resident weights/biases ---
            sw1 = cpool.tile([M, K, DK], dt_mm)
            nc.sync.dma_start(sw1[:], w1t.rearrange("k m d -> m k d"))
            sw2a = cpool.tile([DA, K, L], dt_mm)
            nc.sync.dma_start(sw2a[:], w2t[:, 0:DA, :].rearrange("k d l -> d k l"))
            sw2b = cpool.tile([DB, K, L], dt_mm)
            nc.sync.dma_start(sw2b[:], w2t[:, DA:DK, :].rearrange("k d l -> d k l"))
            sbias = cpool.tile([128, K, 6], f32)
            nc.sync.dma_start(sbias[:], bias[:])

            def bias_col(k, c, p=128):
                return sbias[0:p, k, c : c + 1]

            import contextlib
            loop_cm = tc.For_i(0, repeat, 1, hint_engines=(mybir.EngineType.PE,)) \
                if repeat > 1 else contextlib.nullcontext()
            with loop_cm:
              for k in range(K):
                  sx = xpool.tile([M, BL], dt_mm, tag="sx")
                  nc.sync.dma_start(sx[:], xT[k * M : (k + 1) * M, :])
                  so = opool.tile([L, BL], o_dt, tag="so")
                  w1a = sw1[:, k, 0:DA]
                  w1b = sw1[:, k, DA:DK]
                  w2a = sw2a[:, k, :]
                  w2b = sw2b[:, k, :]
                  for h in range(n_half):
                      hs = slice(h * BT, (h + 1) * BT)
                      pha = psum.tile([DA, BT], f32, tag="pha", bufs=1)
                      phb = psum.tile([DB, BT], f32, tag="phb", bufs=1)
                      po = psum.tile([L, BT], f32, tag="po", bufs=2)
                      for i in range(n_mm):
                          ms = slice(h * BT + i * NMM, h * BT + (i + 1) * NMM)
                          ps = slice(i * NMM, (i + 1) * NMM)
                          nc.tensor.matmul(pha[:, ps], lhsT=w1a, rhs=sx[:, ms], start=True, stop=True)
                          nc.tensor.matmul(phb[:, ps], lhsT=w1b, rhs=sx[:, ms], start=True, stop=True)
                      # leaky(z) for chunk A on ScalarE (Prelu: z>0 ? z : alpha*z)
                      sha = hpool.tile([DA, BT], dt_mm, tag="sha")
                      nc.scalar.activation(sha[:], pha[:], A.Prelu,
                                           bias=bias_col(k, 0), scale=1.0, alpha=SLOPE)
                      # leaky(z) for chunk B on VectorE: t = 0.2*psum + 0.2*b1 ; max(psum + b1, t)
                      tb = hpool.tile([DB, BT], f32, tag="tb")
                      nc.vector.tensor_scalar(tb[:], phb[:], SLOPE, bias_col(k, 3, DB),
                                              OP.mult, OP.add)
                      shb = hpool.tile([DB, BT], dt_mm, tag="shb")
                      nc.vector.scalar_tensor_tensor(shb[:], phb[:], bias_col(k, 1, DB), tb[:],
                                                     OP.add, OP.max)
                      for i in range(n_mm):
                          ps = slice(i * NMM, (i + 1) * NMM)
                          nc.tensor.matmul(po[:, ps], lhsT=w2a, rhs=sha[:, ps], start=True, stop=False)
                          nc.tensor.matmul(po[:, ps], lhsT=w2b, rhs=shb[:, ps], start=False, stop=True)
                      nc.scalar.activation(so[:, hs], po[:], A.Prelu,
                                           bias=bias_col(k, 4), scale=1.0, alpha=SLOPE)
                  nc.sync.dma_start(oT[k * L : (k + 1) * L, :], so[:])
    nc.compile()
    return nc


def _prep(x, W1, b1, W2, b2, dt_name):
    """Host-side shard + layout prep."""
    _, np_dt = _DTYPES[dt_name]
    # diagonal W1 blocks: [k, d, m] -> lhsT [k, m, d]
    W1r = W1.reshape(K, DK, K, M)
    idx = np.arange(K)
    W1d = W1r[idx, :, idx, :]                       # [k, d, m]
    w1t = np.ascontiguousarray(W1d.transpose(0, 2, 1)).astype(np_dt)   # [k, m, d]
    w2t = np.ascontiguousarray(W2.transpose(0, 2, 1)).astype(np_dt)    # [k, d, l]
    bias = np.zeros((128, K, 6), np.float32)
    bias[:, :, 0] = b1[:, 0:DA].T
    bias[0:DB, :, 1] = b1[:, DA:DK].T
    bias[:, :, 2] = SLOPE * b1[:, 0:DA].T
    bias[0:DB, :, 3] = SLOPE * b1[:, DA:DK].T
    bias[:, :, 4] = b2.T
    bias[:, :, 5] = SLOPE * b2.T

    in_maps = []
    for c in range(NCORES):
        xT = np.ascontiguousarray(x[c * BL : (c + 1) * BL, :].T).astype(np_dt)
        in_maps.append({"xT": xT, "w1t": w1t, "w2t": w2t, "bias": bias})
    return in_maps



# ---------------- Variant C: fold 0.2*W2*z into a precomputed Wc matmul ----
# leaky(z) = 0.8*relu(z) + 0.2*z, so with r = relu(W1 x + b1):
#   o_pre = W2 leaky(z) + b2 = (0.2 W2 W1) x + (0.8 W2) r + (b2 + 0.2 W2 b1)
# The Wc = 0.2*W2*W1 matmul streams straight from x (no activation dep),
# r needs only a single relu op per element, and the output activation is a
# single 2048-wide Prelu per expert.

RA_ACT_FRAC = 0.5   # fraction of rA tiles on ScalarE (rest on VectorE)
NBT = 1024          # activation tile width
XBUFS = 2
OBUFS = 2
OUT_BF16 = False
PHBUFS = 1
POBUFS = 2
NMM = 512   # matmul moving free dim (fp32/fp32r hard cap 512)


def _build_c(dt_name, repeat=1):
    dt_mm, _ = _DTYPES[dt_name]
    f32 = mybir.dt.float32
    nc = bacc.Bacc("TRN2", target_bir_lowering=False, debug=False, num_devices=NCORES)

    xT = nc.dram_tensor("xT", [K * M, BL], dt_mm, kind="ExternalInput")
    w1t = nc.dram_tensor("w1t", [K, M, DK], dt_mm, kind="ExternalInput")    # [k][m, d]
    w2r = nc.dram_tensor("w2r", [K, DK, L], dt_mm, kind="ExternalInput")    # 0.8*W2, [k][d, l]
    wc = nc.dram_tensor("wc", [K, M, L], dt_mm, kind="ExternalInput")       # 0.2*(W2@W1d).T, [k][m, l]
    bias = nc.dram_tensor("bias", [128, K, 3], f32, kind="ExternalInput")
    o_dt = mybir.dt.bfloat16 if OUT_BF16 else f32
    oT = nc.dram_tensor("oT", [K * L, BL], o_dt, kind="ExternalOutput")

    nmm = NMM if dt_name == "bf16" else min(NMM, 512)   # fp32 moving-dim limit
    n_bt = BL // NBT            # activation tiles per expert
    n_mm = NBT // nmm           # matmuls per activation tile

    with TileContext(nc) as tc:
        with (
            tc.tile_pool(name="const", bufs=1) as cpool,
            tc.tile_pool(name="xin", bufs=XBUFS) as xpool,
            tc.tile_pool(name="h", bufs=2) as hpool,
            tc.tile_pool(name="o", bufs=OBUFS) as opool,
            tc.tile_pool(name="psum", bufs=1, space="PSUM") as psum,
        ):
            sw1 = cpool.tile([M, K, DK], dt_mm)
            sw2a = cpool.tile([DA, K, L], dt_mm)
            sw2b = cpool.tile([DB, K, L], dt_mm)
            swc = cpool.tile([M, K, L], dt_mm)
            sbias = cpool.tile([128, K, 3], f32)

            def bias_col(k, c, p=128):
                return sbias[0:p, k, c : c + 1]

            import contextlib
            loop_cm = tc.For_i(0, repeat, 1, hint_engines=(mybir.EngineType.PE,)) \
                if repeat > 1 else contextlib.nullcontext()
            with loop_cm:
              ract = 0
              for k in range(K):
                sx = xpool.tile([M, BL], dt_mm, tag="sx")
                nc.sync.dma_start(sx[:], xT[k * M : (k + 1) * M, :])
                if k == 0:
                    # bulk weight loads right after x(0) so expert 0 starts fast
                    nc.sync.dma_start(sw1[:], w1t.rearrange("k m d -> m k d"))
                    nc.sync.dma_start(swc[:], wc.rearrange("k m l -> m k l"))
                    nc.sync.dma_start(sw2a[:], w2r[:, 0:DA, :].rearrange("k d l -> d k l"))
                    nc.sync.dma_start(sw2b[:], w2r[:, DA:DK, :].rearrange("k d l -> d k l"))
                    nc.sync.dma_start(sbias[:], bias[:])
                so = opool.tile([L, BL], o_dt, tag="so")
                w1a = sw1[:, k, 0:DA]
                w1b = sw1[:, k, DA:DK]
                w2a = sw2a[:, k, :]
                w2b = sw2b[:, k, :]
                wck = swc[:, k, :]
                r_dt = f32 if dt_name == "fp32" else dt_mm
                ra = hpool.tile([DA, BL], r_dt, tag="ra")
                rb = hpool.tile([DB, BL], r_dt, tag="rb")
                po_tiles = {}

                def stage1(j):
                    nonlocal ract
                    pha = psum.tile([DA, NBT], f32, tag="pha", bufs=PHBUFS)
                    phb = psum.tile([DB, NBT], f32, tag="phb", bufs=PHBUFS)
                    for i in range(n_mm):
                        ms = slice(j * NBT + i * nmm, j * NBT + (i + 1) * nmm)
                        pp = slice(i * nmm, (i + 1) * nmm)
                        nc.tensor.matmul(pha[:, pp], lhsT=w1a, rhs=sx[:, ms], start=True, stop=True)
                        nc.tensor.matmul(phb[:, pp], lhsT=w1b, rhs=sx[:, ms], start=True, stop=True)
                    ps = slice(j * NBT, (j + 1) * NBT)
                    if (ract * 977) % 1000 < RA_ACT_FRAC * 1000:
                        nc.scalar.activation(ra[:, ps], pha[:], A.Relu,
                                             bias=bias_col(k, 0), scale=1.0)
                    else:
                        nc.vector.tensor_scalar(ra[:, ps], pha[:], bias_col(k, 0), 0.0,
                                                OP.add, OP.max)
                    ract += 1
                    nc.vector.tensor_scalar(rb[:, ps], phb[:], bias_col(k, 1, DB), 0.0,
                                            OP.add, OP.max)

                def stage2(j):
                    po = psum.tile([L, NBT], f32, tag="po", bufs=POBUFS, name=f"po_{k}_{j}")
                    for i in range(n_mm):
                        ms = slice(j * NBT + i * nmm, j * NBT + (i + 1) * nmm)
                        pp = slice(i * nmm, (i + 1) * nmm)
                        nc.tensor.matmul(po[:, pp], lhsT=wck, rhs=sx[:, ms], start=True, stop=False)
                        nc.tensor.matmul(po[:, pp], lhsT=w2a, rhs=ra[:, ms], start=False, stop=False)
                        nc.tensor.matmul(po[:, pp], lhsT=w2b, rhs=rb[:, ms], start=False, stop=True)
                    ps = slice(j * NBT, (j + 1) * NBT)
                    nc.scalar.activation(so[:, ps], po[:], A.Prelu,
                                         bias=bias_col(k, 2), scale=1.0, alpha=SLOPE)

                for j in range(n_bt + 1):
                    if j < n_bt:
                        stage1(j)
                    if j >= 1:
                        stage2(j - 1)
                nc.sync.dma_start(oT[k * L : (k + 1) * L, :], so[:])
    nc.compile()
    return nc


def _prep_c(x, W1, b1, W2, b2, dt_name):
    _, np_dt = _DTYPES[dt_name]
    W1r = W1.reshape(K, DK, K, M)
    idx = np.arange(K)
    W1d = W1r[idx, :, idx, :]                                            # [k, d, m]
    w1t = np.ascontiguousarray(W1d.transpose(0, 2, 1)).astype(np_dt)     # [k, m, d]
    w2r = np.ascontiguousarray((0.8 * W2).transpose(0, 2, 1)).astype(np_dt)
    wck = 0.2 * np.matmul(W2, W1d)                                       # [k, l, m]
    wc = np.ascontiguousarray(wck.transpose(0, 2, 1)).astype(np_dt)      # [k, m, l]
    b2p = b2 + 0.2 * np.einsum("kld,kd->kl", W2, b1)
    bias = np.zeros((128, K, 3), np.float32)
    bias[:, :, 0] = b1[:, 0:DA].T
    bias[0:DB, :, 1] = b1[:, DA:DK].T
    bias[:, :, 2] = b2p.T
    in_maps = []
    for c in range(NCORES):
        xTc = np.ascontiguousarray(x[c * BL : (c + 1) * BL, :].T).astype(np_dt)
        in_maps.append({"xT": xTc, "w1t": w1t, "w2r": w2r, "wc": wc, "bias": bias})
    return in_maps


VARIANT = "c"   # "a" | "c"

def _run(x, W1, b1, W2, b2, repeat=1):
    x, W1, b1, W2, b2 = (np.asarray(a) for a in (x, W1, b1, W2, b2))
    key = (VARIANT, DT, repeat)
    if key not in _cache:
        _cache[key] = (_build_c if VARIANT == "c" else _build)(DT, repeat)
    nc = _cache[key]
    in_maps = (_prep_c if VARIANT == "c" else _prep)(x, W1, b1, W2, b2, DT)
    res = run_bass_kernel_spmd(nc, in_maps, list(range(NCORES)))
    out = np.empty((B, K * L), np.float32)
    for c in range(NCORES):
        out[c * BL : (c + 1) * BL, :] = res.results[c]["oT"].T.astype(np.float32)
    return out, res


def kernel(x, W1, b1, W2, b2):
    out, _ = _run(x, W1, b1, W2, b2)
    return out


def measure_hw_time(x, W1, b1, W2, b2, repeat=(10000, 20000), rounds=3):
    """Estimate per-pass on-device time: the kernel body runs inside a hardware
    For_i loop; per-pass time = slope of wall-clock between two large repeat
    counts (transfer/dispatch overheads cancel; axon tunnel noise ~0.1s forces
    large R)."""
    import time as _time
    in_maps = (_prep_c if VARIANT == "c" else _prep)(x, W1, b1, W2, b2, DT)
    r_lo, r_hi = repeat
    walls = {}
    for r in (r_lo, r_hi):
        key = (VARIANT, DT, r)
        if key not in _cache:
            _cache[key] = (_build_c if VARIANT == "c" else _build)(DT, r)
        nc = _cache[key]
        run_bass_kernel_spmd(nc, in_maps, list(range(NCORES)))  # warm (jit compile)
        best = float("inf")
        for _ in range(rounds):
            t0 = _time.perf_counter()
            run_bass_kernel_spmd(nc, in_maps, list(range(NCORES)))
            best = min(best, _time.perf_counter() - t0)
        walls[r] = best
    hw_s = (walls[r_hi] - walls[r_lo]) / (r_hi - r_lo)
    return hw_s * 1e9, walls



# revision 4
# speedup vs baseline: 1.0201x; 1.0201x over previous
"""Trainium2 Bass kernel for nn_CompositionalMLP_75763223101514.

Reference computation (per batch row b, expert k):
    xb = x.reshape(B, 16, 128)
    h  = leaky( einsum('bkm,kdm->bkd', xb, W1diag) + b1 )    # W1diag[k] = W1[k,:,k*128:(k+1)*128]
    o  = leaky( einsum('bkd,kld->bkl', h, W2) + b2 )
    out = o.reshape(B, 16*128)
with leaky(z) = z if z > 0 else 0.2 z.

Strategy: data-parallel over the batch dim across 8 NeuronCores (2048 rows
each), weights replicated.  On the host we pre-transpose each x shard to
feature-major [2048, 2048] so the contraction dim (m) lands on SBUF
partitions, extract the diagonal W1 blocks, and pre-transpose the weights
into lhsT layout.  Each core then runs, per expert k:

    MM1:  hT[d, b]  = sum_m W1T_k[m, d] * xT[k*128+m, b]     (PE, contraction 128)
    act:  h = leaky(hT + b1)  (ScalarE Prelu for the d<128 chunk + the output;
                               VectorE 2-op max(z, 0.2z) for the d>=128 chunk)
    MM2:  oT[l, b]  = sum_d W2T_k[d, l] * h[d, b]            (PE, contraction 240, accumulated)
    act:  o = leaky(oT + b2)  -> SBUF -> DMA to oT dram [k*128+l, b]

The host finally re-transposes each core's oT shard back to [2048, 2048]
batch-major and concatenates.

Matmul dtype: float32r (single-pass fp32 on the PE at full bf16 rate for
moving dim >= 256; measured max rel err ~1.4e-4 per matmul vs 2.5e-3 for
bf16).  Set DT = "bf16" to halve input DMA instead.
"""

import numpy as np
import ml_dtypes

import concourse.bacc as bacc
import concourse.mybir as mybir
from concourse.tile import TileContext
from concourse.bass_utils import run_bass_kernel_spmd

K, M, DK, L = 16, 128, 240, 128
B = 16384
NCORES = 8
BL = B // NCORES          # batch rows per core
SLOPE = 0.2
DA, DB = 128, DK - 128    # hidden split (PSUM partition limit)

DT = "bf16"               # "bf16" | "fp32r" | "fp32"
BT = 1024                 # activation tile width (columns of local batch)

_DTYPES = {
    "bf16": (mybir.dt.bfloat16, ml_dtypes.bfloat16),
    "fp32r": (mybir.dt.float32r, np.float32),
    "fp32": (mybir.dt.float32, np.float32),
}

A = mybir.ActivationFunctionType
OP = mybir.AluOpType

_cache = {}


def _build(dt_name, repeat=1):
    """One SPMD program; all cores run it on their own batch shard."""
    dt_mm, _ = _DTYPES[dt_name]
    f32 = mybir.dt.float32
    nc = bacc.Bacc("TRN2", target_bir_lowering=False, debug=False, num_devices=NCORES)

    xT = nc.dram_tensor("xT", [K * M, BL], dt_mm, kind="ExternalInput")
    w1t = nc.dram_tensor("w1t", [K, M, DK], dt_mm, kind="ExternalInput")   # [k][m, d]
    w2t = nc.dram_tensor("w2t", [K, DK, L], dt_mm, kind="ExternalInput")   # [k][d, l]
    # bias pack: [:, k, 0]=b1[:128]  [:112, k, 1]=b1[128:]  [:, k, 2]=0.2*b1[:128]
    #            [:112, k, 3]=0.2*b1[128:]  [:, k, 4]=b2  [:, k, 5]=0.2*b2
    bias = nc.dram_tensor("bias", [128, K, 6], f32, kind="ExternalInput")
    oT = nc.dram_tensor("oT", [K * L, BL], f32, kind="ExternalOutput")

    n_half = BL // BT           # halves per expert
    n_mm = BT // NMM            # matmuls per half per chunk

    with TileContext(nc) as tc:
        with (
            tc.tile_pool(name="const", bufs=1) as cpool,
            tc.tile_pool(name="xin", bufs=XBUFS) as xpool,
            tc.tile_pool(name="h", bufs=2) as hpool,
            tc.tile_pool(name="o", bufs=OBUFS) as opool,
            tc.tile_pool(name="psum", bufs=1, space="PSUM") as psum,
        ):
            # --- resident weights/biases ---
            sw1 = cpool.tile([M, K, DK], dt_mm)
            nc.sync.dma_start(sw1[:], w1t.rearrange("k m d -> m k d"))
            sw2a = cpool.tile([DA, K, L], dt_mm)
            nc.sync.dma_start(sw2a[:], w2t[:, 0:DA, :].rearrange("k d l -> d k l"))
            sw2b = cpool.tile([DB, K, L], dt_mm)
            nc.sync.dma_start(sw2b[:], w2t[:, DA:DK, :].rearrange("k d l -> d k l"))
            sbias = cpool.tile([128, K, 6], f32)
            nc.sync.dma_start(sbias[:], bias[:])

            def bias_col(k, c, p=128):
                return sbias[0:p, k, c : c + 1]

            import contextlib
            loop_cm = tc.For_i(0, repeat, 1, hint_engines=(mybir.EngineType.PE,)) \
                if repeat > 1 else contextlib.nullcontext()
            with loop_cm:
              for k in range(K):
                  sx = xpool.tile([M, BL], dt_mm, tag="sx")
                  nc.sync.dma_start(sx[:], xT[k * M : (k + 1) * M, :])
                  so = opool.tile([L, BL], o_dt, tag="so")
                  w1a = sw1[:, k, 0:DA]
                  w1b = sw1[:, k, DA:DK]
                  w2a = sw2a[:, k, :]
                  w2b = sw2b[:, k, :]
                  for h in range(n_half):
                      hs = slice(h * BT, (h + 1) * BT)
                      pha = psum.tile([DA, BT], f32, tag="pha", bufs=1)
                      phb = psum.tile([DB, BT], f32, tag="phb", bufs=1)
                      po = psum.tile([L, BT], f32, tag="po", bufs=2)
                      for i in range(n_mm):
                          ms = slice(h * BT + i * NMM, h * BT + (i + 1) * NMM)
                          ps = slice(i * NMM, (i + 1) * NMM)
                          nc.tensor.matmul(pha[:, ps], lhsT=w1a, rhs=sx[:, ms], start=True, stop=True)
                          nc.tensor.matmul(phb[:, ps], lhsT=w1b, rhs=sx[:, ms], start=True, stop=True)
                      # leaky(z) for chunk A on ScalarE (Prelu: z>0 ? z : alpha*z)
                      sha = hpool.tile([DA, BT], dt_mm, tag="sha")
                      nc.scalar.activation(sha[:], pha[:], A.Prelu,
                                           bias=bias_col(k, 0), scale=1.0, alpha=SLOPE)
                      # leaky(z) for chunk B on VectorE: t = 0.2*psum + 0.2*b1 ; max(psum + b1, t)
                      tb = hpool.tile([DB, BT], f32, tag="tb")
                      nc.vector.tensor_scalar(tb[:], phb[:], SLOPE, bias_col(k, 3, DB),
                                              OP.mult, OP.add)
                      shb = hpool.tile([DB, BT], dt_mm, tag="shb")
                      nc.vector.scalar_tensor_tensor(shb[:], phb[:], bias_col(k, 1, DB), tb[:],
                                                     OP.add, OP.max)
                      for i in range(n_mm):
                          ps = slice(i * NMM, (i + 1) * NMM)
                          nc.tensor.matmul(po[:, ps], lhsT=w2a, rhs=sha[:, ps], start=True, stop=False)
                          nc.tensor.matmul(po[:, ps], lhsT=w2b, rhs=shb[:, ps], start=False, stop=True)
                      nc.scalar.activation(so[:, hs], po[:], A.Prelu,
                                           bias=bias_col(k, 4), scale=1.0, alpha=SLOPE)
                  nc.sync.dma_start(oT[k * L : (k + 1) * L, :], so[:])
    nc.compile()
    return nc


def _prep(x, W1, b1, W2, b2, dt_name):
    """Host-side shard + layout prep."""
    _, np_dt = _DTYPES[dt_name]
    # diagonal W1 blocks: [k, d, m] -> lhsT [k, m, d]
    W1r = W1.reshape(K, DK, K, M)
    idx = np.arange(K)
    W1d = W1r[idx, :, idx, :]                       # [k, d, m]
    w1t = np.ascontiguousarray(W1d.transpose(0, 2, 1)).astype(np_dt)   # [k, m, d]
    w2t = np.ascontiguousarray(W2.transpose(0, 2, 1)).astype(np_dt)    # [k, d, l]
    bias = np.zeros((128, K, 6), np.float32)
    bias[:, :, 0] = b1[:, 0:DA].T
    bias[0:DB, :, 1] = b1[:, DA:DK].T
    bias[:, :, 2] = SLOPE * b1[:, 0:DA].T
    bias[0:DB, :, 3] = SLOPE * b1[:, DA:DK].T
    bias[:, :, 4] = b2.T
    bias[:, :, 5] = SLOPE * b2.T

    in_maps = []
    for c in range(NCORES):
        xT = np.ascontiguousarray(x[c * BL : (c + 1) * BL, :].T).astype(np_dt)
        in_maps.append({"xT": xT, "w1t": w1t, "w2t": w2t, "bias": bias})
    return in_maps



# ---------------- Variant C: fold 0.2*W2*z into a precomputed Wc matmul ----
# leaky(z) = 0.8*relu(z) + 0.2*z, so with r = relu(W1 x + b1):
#   o_pre = W2 leaky(z) + b2 = (0.2 W2 W1) x + (0.8 W2) r + (b2 + 0.2 W2 b1)
# The Wc = 0.2*W2*W1 matmul streams straight from x (no activation dep),
# r needs only a single relu op per element, and the output activation is a
# single 2048-wide Prelu per expert.

RA_ACT_FRAC = 0.5   # fraction of rA tiles on ScalarE (rest on VectorE)
NBT = 1024          # activation tile width
XBUFS = 2
OBUFS = 2
OUT_BF16 = True
PHBUFS = 1
POBUFS = 2
NMM = 512   # matmul moving free dim (fp32/fp32r hard cap 512)


def _build_c(dt_name, repeat=1):
    dt_mm, _ = _DTYPES[dt_name]
    f32 = mybir.dt.float32
    nc = bacc.Bacc("TRN2", target_bir_lowering=False, debug=False, num_devices=NCORES)

    xT = nc.dram_tensor("xT", [K * M, BL], dt_mm, kind="ExternalInput")
    w1t = nc.dram_tensor("w1t", [K, M, DK], dt_mm, kind="ExternalInput")    # [k][m, d]
    w2r = nc.dram_tensor("w2r", [K, DK, L], dt_mm, kind="ExternalInput")    # 0.8*W2, [k][d, l]
    wc = nc.dram_tensor("wc", [K, M, L], dt_mm, kind="ExternalInput")       # 0.2*(W2@W1d).T, [k][m, l]
    bias = nc.dram_tensor("bias", [128, K, 3], f32, kind="ExternalInput")
    o_dt = mybir.dt.bfloat16 if OUT_BF16 else f32
    oT = nc.dram_tensor("oT", [K * L, BL], o_dt, kind="ExternalOutput")

    nmm = NMM if dt_name == "bf16" else min(NMM, 512)   # fp32 moving-dim limit
    n_bt = BL // NBT            # activation tiles per expert
    n_mm = NBT // nmm           # matmuls per activation tile

    with TileContext(nc) as tc:
        with (
            tc.tile_pool(name="const", bufs=1) as cpool,
            tc.tile_pool(name="xin", bufs=XBUFS) as xpool,
            tc.tile_pool(name="h", bufs=2) as hpool,
            tc.tile_pool(name="o", bufs=OBUFS) as opool,
            tc.tile_pool(name="psum", bufs=1, space="PSUM") as psum,
        ):
            sw1 = cpool.tile([M, K, DK], dt_mm)
            sw2a = cpool.tile([DA, K, L], dt_mm)
            sw2b = cpool.tile([DB, K, L], dt_mm)
            swc = cpool.tile([M, K, L], dt_mm)
            sbias = cpool.tile([128, K, 3], f32)

            def bias_col(k, c, p=128):
                return sbias[0:p, k, c : c + 1]

            import contextlib
            loop_cm = tc.For_i(0, repeat, 1, hint_engines=(mybir.EngineType.PE,)) \
                if repeat > 1 else contextlib.nullcontext()
            with loop_cm:
              ract = 0
              for k in range(K):
                sx = xpool.tile([M, BL], dt_mm, tag="sx")
                nc.sync.dma_start(sx[:], xT[k * M : (k + 1) * M, :])
                if k == 0:
                    # bulk weight loads right after x(0) so expert 0 starts fast
                    nc.sync.dma_start(sw1[:], w1t.rearrange("k m d -> m k d"))
                    nc.sync.dma_start(swc[:], wc.rearrange("k m l -> m k l"))
                    nc.sync.dma_start(sw2a[:], w2r[:, 0:DA, :].rearrange("k d l -> d k l"))
                    nc.sync.dma_start(sw2b[:], w2r[:, DA:DK, :].rearrange("k d l -> d k l"))
                    nc.sync.dma_start(sbias[:], bias[:])
                so = opool.tile([L, BL], o_dt, tag="so")
                w1a = sw1[:, k, 0:DA]
                w1b = sw1[:, k, DA:DK]
                w2a = sw2a[:, k, :]
                w2b = sw2b[:, k, :]
                wck = swc[:, k, :]
                r_dt = f32 if dt_name == "fp32" else dt_mm
                ra = hpool.tile([DA, BL], r_dt, tag="ra")
                rb = hpool.tile([DB, BL], r_dt, tag="rb")
                po_tiles = {}

                def stage1(j):
                    nonlocal ract
                    pha = psum.tile([DA, NBT], f32, tag="pha", bufs=PHBUFS)
                    phb = psum.tile([DB, NBT], f32, tag="phb", bufs=PHBUFS)
                    for i in range(n_mm):
                        ms = slice(j * NBT + i * nmm, j * NBT + (i + 1) * nmm)
                        pp = slice(i * nmm, (i + 1) * nmm)
                        nc.tensor.matmul(pha[:, pp], lhsT=w1a, rhs=sx[:, ms], start=True, stop=True)
                        nc.tensor.matmul(phb[:, pp], lhsT=w1b, rhs=sx[:, ms], start=True, stop=True)
                    ps = slice(j * NBT, (j + 1) * NBT)
                    if (ract * 977) % 1000 < RA_ACT_FRAC * 1000:
                        nc.scalar.activation(ra[:, ps], pha[:], A.Relu,
                                             bias=bias_col(k, 0), scale=1.0)
                    else:
                        nc.vector.tensor_scalar(ra[:, ps], pha[:], bias_col(k, 0), 0.0,
                                                OP.add, OP.max)
                    ract += 1
                    nc.vector.tensor_scalar(rb[:, ps], phb[:], bias_col(k, 1, DB), 0.0,
                                            OP.add, OP.max)

                def stage2(j):
                    po = psum.tile([L, NBT], f32, tag="po", bufs=POBUFS, name=f"po_{k}_{j}")
                    for i in range(n_mm):
                        ms = slice(j * NBT + i * nmm, j * NBT + (i + 1) * nmm)
                        pp = slice(i * nmm, (i + 1) * nmm)
                        nc.tensor.matmul(po[:, pp], lhsT=wck, rhs=sx[:, ms], start=True, stop=False)
                        nc.tensor.matmul(po[:, pp], lhsT=w2a, rhs=ra[:, ms], start=False, stop=False)
                        nc.tensor.matmul(po[:, pp], lhsT=w2b, rhs=rb[:, ms], start=False, stop=True)
                    ps = slice(j * NBT, (j + 1) * NBT)
                    nc.scalar.activation(so[:, ps], po[:], A.Prelu,
                                         bias=bias_col(k, 2), scale=1.0, alpha=SLOPE)

                for j in range(n_bt + 1):
                    if j < n_bt:
                        stage1(j)
                    if j >= 1:
                        stage2(j - 1)
                nc.sync.dma_start(oT[k * L : (k + 1) * L, :], so[:])
    nc.compile()
    return nc


def _prep_c(x, W1, b1, W2, b2, dt_name):
    _, np_dt = _DTYPES[dt_name]
    W1r = W1.reshape(K, DK, K, M)
    idx = np.arange(K)
    W1d = W1r[idx, :, idx, :]                                            # [k, d, m]
    w1t = np.ascontiguousarray(W1d.transpose(0, 2, 1)).astype(np_dt)     # [k, m, d]
    w2r = np.ascontiguousarray((0.8 * W2).transpose(0, 2, 1)).astype(np_dt)
    wck = 0.2 * np.matmul(W2, W1d)                                       # [k, l, m]
    wc = np.ascontiguousarray(wck.transpose(0, 2, 1)).astype(np_dt)      # [k, m, l]
    b2p = b2 + 0.2 * np.einsum("kld,kd->kl", W2, b1)
    bias = np.zeros((128, K, 3), np.float32)
    bias[:, :, 0] = b1[:, 0:DA].T
    bias[0:DB, :, 1] = b1[:, DA:DK].T
    bias[:, :, 2] = b2p.T
    in_maps = []
    for c in range(NCORES):
        xTc = np.ascontiguousarray(x[c * BL : (c + 1) * BL, :].T).astype(np_dt)
        in_maps.append({"xT": xTc, "w1t": w1t, "w2r": w2r, "wc": wc, "bias": bias})
    return in_maps


# ---------------- Variant D: 4 matmuls/tile, true leaky on h ---------------
# Drops variant C's extra Wc matmul (PE 5 -> 4 N-passes per tile per expert:
# 54.6us/core floor) and instead computes leaky(h) directly, spread across
# ScalarE (1-pass Prelu), VectorE (PSUM pass + SBUF 2x-mode max pass) and
# GpSimd/Pool (2 passes @1.2GHz).  Weights stream per-expert (bufs=2) so
# startup and cross-iteration weight DMA overlap compute.

NBT_D = 512           # tile width (1 PSUM bank per [*,512] f32 tile)
HB_PAT = ("v", "p")   # engine cycle for the hB leaky
O_PAT = ("s", "v", "p")   # engine cycle for the output leaky
HA_PAT = ("s",)       # engine cycle for the hA leaky


def _build_d(dt_name, repeat=1):
    dt_mm, _ = _DTYPES[dt_name]
    f32 = mybir.dt.float32
    bf16 = mybir.dt.bfloat16
    nc = bacc.Bacc("TRN2", target_bir_lowering=False, debug=False, num_devices=NCORES)

    xT = nc.dram_tensor("xT", [K * M, BL], dt_mm, kind="ExternalInput")
    w1t = nc.dram_tensor("w1t", [K, M, DK], dt_mm, kind="ExternalInput")    # [k][m, d]
    w2t = nc.dram_tensor("w2t", [K, DK, L], dt_mm, kind="ExternalInput")    # [k][d, l]
    bias = nc.dram_tensor("bias", [128, K, 3], f32, kind="ExternalInput")
    o_dt = bf16 if OUT_BF16 else f32
    oT = nc.dram_tensor("oT", [K * L, BL], o_dt, kind="ExternalOutput")

    n_bt = BL // NBT_D
    r_dt = bf16 if dt_name != "fp32" else f32

    with TileContext(nc) as tc:
        with (
            tc.tile_pool(name="const", bufs=1) as cpool,
            tc.tile_pool(name="wts", bufs=2) as wpool,
            tc.tile_pool(name="xin", bufs=XBUFS) as xpool,
            tc.tile_pool(name="h", bufs=2) as hpool,
            tc.tile_pool(name="u", bufs=3) as upool,
            tc.tile_pool(name="o", bufs=OBUFS) as opool,
            tc.tile_pool(name="psum", bufs=1, space="PSUM") as psum,
        ):
            sbias = cpool.tile([128, K, 3], f32)
            nc.sync.dma_start(sbias[:], bias[:])

            def bias_col(k, c, p=128):
                return sbias[0:p, k, c : c + 1]

            import contextlib
            loop_cm = tc.For_i(0, repeat, 1, hint_engines=(mybir.EngineType.PE,)) \
                if repeat > 1 else contextlib.nullcontext()
            with loop_cm:
              counts = {"hb": 0, "o": 0, "ha": 0}

              def leaky(eng, dst, src_psum, bcol, p):
                  """dst = leaky(src_psum + bias) via the chosen engine."""
                  if eng == "s":
                      nc.scalar.activation(dst, src_psum, A.Prelu,
                                           bias=bcol, scale=1.0, alpha=SLOPE)
                  else:
                      e = nc.vector if eng == "v" else nc.gpsimd
                      u = upool.tile([p, NBT_D], r_dt, tag="u" + eng)
                      e.tensor_scalar(u[:], src_psum, bcol, None, OP.add, OP.bypass)
                      e.scalar_tensor_tensor(dst, u[:], SLOPE, u[:], OP.mult, OP.max)

              for k in range(K):
                  sx = xpool.tile([M, BL], dt_mm, tag="sx")
                  nc.sync.dma_start(sx[:], xT[k * M : (k + 1) * M, :])
                  sw1 = wpool.tile([M, DK], dt_mm, tag="sw1")
                  nc.sync.dma_start(sw1[:], w1t[k])
                  sw2a = wpool.tile([DA, L], dt_mm, tag="sw2a")
                  nc.sync.dma_start(sw2a[:], w2t[k, 0:DA, :])
                  sw2b = wpool.tile([DB, L], dt_mm, tag="sw2b")
                  nc.sync.dma_start(sw2b[:], w2t[k, DA:DK, :])

                  so = opool.tile([L, BL], o_dt, tag="so")
                  ra = hpool.tile([DA, BL], r_dt, tag="ra")
                  rb = hpool.tile([DB, BL], r_dt, tag="rb")
                  w1a = sw1[:, 0:DA]
                  w1b = sw1[:, DA:DK]

                  def stage1(j):
                      ms = slice(j * NBT_D, (j + 1) * NBT_D)
                      pha = psum.tile([DA, NBT_D], f32, tag="pha", bufs=2)
                      phb = psum.tile([DB, NBT_D], f32, tag="phb", bufs=2)
                      nc.tensor.matmul(pha[:], lhsT=w1a, rhs=sx[:, ms], start=True, stop=True)
                      nc.tensor.matmul(phb[:], lhsT=w1b, rhs=sx[:, ms], start=True, stop=True)
                      ea = HA_PAT[counts["ha"] % len(HA_PAT)]; counts["ha"] += 1
                      leaky(ea, ra[:, ms], pha[:], bias_col(k, 0), DA)
                      eb = HB_PAT[counts["hb"] % len(HB_PAT)]; counts["hb"] += 1
                      leaky(eb, rb[:, ms], phb[:], bias_col(k, 1, DB), DB)

                  def stage2(j):
                      ms = slice(j * NBT_D, (j + 1) * NBT_D)
                      po = psum.tile([L, NBT_D], f32, tag="po", bufs=POBUFS)
                      nc.tensor.matmul(po[:], lhsT=sw2a[:], rhs=ra[:, ms], start=True, stop=False)
                      nc.tensor.matmul(po[:], lhsT=sw2b[:], rhs=rb[:, ms], start=False, stop=True)
                      eo = O_PAT[counts["o"] % len(O_PAT)]; counts["o"] += 1
                      leaky(eo, so[:, ms], po[:], bias_col(k, 2), L)

                  for j in range(n_bt + 1):
                      if j < n_bt:
                          stage1(j)
                      if j >= 1:
                          stage2(j - 1)
                  nc.sync.dma_start(oT[k * L : (k + 1) * L, :], so[:])
    nc.compile()
    return nc


def _prep_d(x, W1, b1, W2, b2, dt_name):
    _, np_dt = _DTYPES[dt_name]
    W1r = W1.reshape(K, DK, K, M)
    idx = np.arange(K)
    W1d = W1r[idx, :, idx, :]                                            # [k, d, m]
    w1t = np.ascontiguousarray(W1d.transpose(0, 2, 1)).astype(np_dt)     # [k, m, d]
    w2t = np.ascontiguousarray(W2.transpose(0, 2, 1)).astype(np_dt)      # [k, d, l]
    bias = np.zeros((128, K, 3), np.float32)
    bias[:, :, 0] = b1[:, 0:DA].T
    bias[0:DB, :, 1] = b1[:, DA:DK].T
    bias[:, :, 2] = b2.T
    in_maps = []
    for c in range(NCORES):
        xTc = np.ascontiguousarray(x[c * BL : (c + 1) * BL, :].T).astype(np_dt)
        in_maps.append({"xT": xTc, "w1t": w1t, "w2t": w2t, "bias": bias})
    return in_maps


_BUILDERS = {"a": (_build, _prep), "c": (_build_c, _prep_c), "d": (_build_d, _prep_d)}

VARIANT = "c"   # "a" | "c" | "d"

def _run(x, W1, b1, W2, b2, repeat=1):
    x, W1, b1, W2, b2 = (np.asarray(a) for a in (x, W1, b1, W2, b2))
    key = (VARIANT, DT, repeat)
    if key not in _cache:
        _cache[key] = (_build_c if VARIANT == "c" else _build)(DT, repeat)
    nc = _cache[key]
    in_maps = (_prep_c if VARIANT == "c" else _prep)(x, W1, b1, W2, b2, DT)
    res = run_bass_kernel_spmd(nc, in_maps, list(range(NCORES)))
    out = np.empty((B, K * L), np.float32)
    for c in range(NCORES):
        out[c * BL : (c + 1) * BL, :] = res.results[c]["oT"].T.astype(np.float32)
    return out, res


def kernel(x, W1, b1, W2, b2):
    out, _ = _run(x, W1, b1, W2, b2)
    return out


def measure_hw_time(x, W1, b1, W2, b2, repeat=(10000, 20000), rounds=3):
    """Estimate per-pass on-device time: the kernel body runs inside a hardware
    For_i loop; per-pass time = slope of wall-clock between two large repeat
    counts (transfer/dispatch overheads cancel; axon tunnel noise ~0.1s forces
    large R)."""
    import time as _time
    in_maps = (_prep_c if VARIANT == "c" else _prep)(x, W1, b1, W2, b2, DT)
    r_lo, r_hi = repeat
    walls = {}
    for r in (r_lo, r_hi):
        key = (VARIANT, DT, r)
        if key not in _cache:
            _cache[key] = (_build_c if VARIANT == "c" else _build)(DT, r)
        nc = _cache[key]
        run_bass_kernel_spmd(nc, in_maps, list(range(NCORES)))  # warm (jit compile)
        best = float("inf")
        for _ in range(rounds):
            t0 = _time.perf_counter()
            run_bass_kernel_spmd(nc, in_maps, list(range(NCORES)))
            best = min(best, _time.perf_counter() - t0)
        walls[r] = best
    hw_s = (walls[r_hi] - walls[r_lo]) / (r_hi - r_lo)
    return hw_s * 1e9, walls



# revision 27
# speedup vs baseline: 1.1040x; 1.0822x over previous
"""Trainium2 Bass kernel for nn_CompositionalMLP_75763223101514.

Reference computation (per batch row b, expert k):
    xb = x.reshape(B, 16, 128)
    h  = leaky( einsum('bkm,kdm->bkd', xb, W1diag) + b1 )    # W1diag[k] = W1[k,:,k*128:(k+1)*128]
    o  = leaky( einsum('bkd,kld->bkl', h, W2) + b2 )
    out = o.reshape(B, 16*128)
with leaky(z) = z if z > 0 else 0.2 z.

Strategy: data-parallel over the batch dim across 8 NeuronCores (2048 rows
each), weights replicated.  On the host we pre-transpose each x shard to
feature-major [2048, 2048] so the contraction dim (m) lands on SBUF
partitions, extract the diagonal W1 blocks, and pre-transpose the weights
into lhsT layout.  Each core then runs, per expert k:

    MM1:  hT[d, b]  = sum_m W1T_k[m, d] * xT[k*128+m, b]     (PE, contraction 128)
    act:  h = leaky(hT + b1)  (ScalarE Prelu for the d<128 chunk + the output;
                               VectorE 2-op max(z, 0.2z) for the d>=128 chunk)
    MM2:  oT[l, b]  = sum_d W2T_k[d, l] * h[d, b]            (PE, contraction 240, accumulated)
    act:  o = leaky(oT + b2)  -> SBUF -> DMA to oT dram [k*128+l, b]

The host finally re-transposes each core's oT shard back to [2048, 2048]
batch-major and concatenates.

Matmul dtype: float32r (single-pass fp32 on the PE at full bf16 rate for
moving dim >= 256; measured max rel err ~1.4e-4 per matmul vs 2.5e-3 for
bf16).  Set DT = "bf16" to halve input DMA instead.
"""

import numpy as np
import ml_dtypes

import concourse.bacc as bacc
import concourse.mybir as mybir
from concourse.tile import TileContext
from concourse.bass_utils import run_bass_kernel_spmd

K, M, DK, L = 16, 128, 240, 128
B = 16384
NCORES = 8
BL = B // NCORES          # batch rows per core
SLOPE = 0.2
DA, DB = 128, DK - 128    # hidden split (PSUM partition limit)

DT = "bf16"               # "bf16" | "fp32r" | "fp32"
BT = 1024                 # activation tile width (columns of local batch)

_DTYPES = {
    "bf16": (mybir.dt.bfloat16, ml_dtypes.bfloat16),
    "fp32r": (mybir.dt.float32r, np.float32),
    "fp32": (mybir.dt.float32, np.float32),
}

A = mybir.ActivationFunctionType
OP = mybir.AluOpType

_cache = {}


def _build(dt_name, repeat=1):
    """One SPMD program; all cores run it on their own batch shard."""
    dt_mm, _ = _DTYPES[dt_name]
    f32 = mybir.dt.float32
    nc = bacc.Bacc("TRN2", target_bir_lowering=False, debug=False, num_devices=NCORES)

    xT = nc.dram_tensor("xT", [K * M, BL], dt_mm, kind="ExternalInput")
    w1t = nc.dram_tensor("w1t", [K, M, DK], dt_mm, kind="ExternalInput")   # [k][m, d]
    w2t = nc.dram_tensor("w2t", [K, DK, L], dt_mm, kind="ExternalInput")   # [k][d, l]
    # bias pack: [:, k, 0]=b1[:128]  [:112, k, 1]=b1[128:]  [:, k, 2]=0.2*b1[:128]
    #            [:112, k, 3]=0.2*b1[128:]  [:, k, 4]=b2  [:, k, 5]=0.2*b2
    bias = nc.dram_tensor("bias", [128, K, 6], f32, kind="ExternalInput")
    oT = nc.dram_tensor("oT", [K * L, BL], f32, kind="ExternalOutput")

    n_half = BL // BT           # halves per expert
    n_mm = BT // NMM            # matmuls per half per chunk

    with TileContext(nc) as tc:
        with (
            tc.tile_pool(name="const", bufs=1) as cpool,
            tc.tile_pool(name="xin", bufs=XBUFS) as xpool,
            tc.tile_pool(name="h", bufs=2) as hpool,
            tc.tile_pool(name="o", bufs=OBUFS) as opool,
            tc.tile_pool(name="psum", bufs=1, space="PSUM") as psum,
        ):
            # --- resident weights/biases ---
            sw1 = cpool.tile([M, K, DK], dt_mm)
            nc.sync.dma_start(sw1[:], w1t.rearrange("k m d -> m k d"))
            sw2a = cpool.tile([DA, K, L], dt_mm)
            nc.sync.dma_start(sw2a[:], w2t[:, 0:DA, :].rearrange("k d l -> d k l"))
            sw2b = cpool.tile([DB, K, L], dt_mm)
            nc.sync.dma_start(sw2b[:], w2t[:, DA:DK, :].rearrange("k d l -> d k l"))
            sbias = cpool.tile([128, K, 6], f32)
            nc.sync.dma_start(sbias[:], bias[:])

            def bias_col(k, c, p=128):
                return sbias[0:p, k, c : c + 1]

            import contextlib
            loop_cm = tc.For_i(0, repeat, 1, hint_engines=(mybir.EngineType.PE,)) \
                if repeat > 1 else contextlib.nullcontext()
            with loop_cm:
              for k in range(K):
                  sx = xpool.tile([M, BL], dt_mm, tag="sx")
                  nc.sync.dma_start(sx[:], xT[k * M : (k + 1) * M, :])
                  so = opool.tile([L, BL], o_dt, tag="so")
                  w1a = sw1[:, k, 0:DA]
                  w1b = sw1[:, k, DA:DK]
                  w2a = sw2a[:, k, :]
                  w2b = sw2b[:, k, :]
                  for h in range(n_half):
                      hs = slice(h * BT, (h + 1) * BT)
                      pha = psum.tile([DA, BT], f32, tag="pha", bufs=1)
                      phb = psum.tile([DB, BT], f32, tag="phb", bufs=1)
                      po = psum.tile([L, BT], f32, tag="po", bufs=2)
                      for i in range(n_mm):
                          ms = slice(h * BT + i * NMM, h * BT + (i + 1) * NMM)
                          ps = slice(i * NMM, (i + 1) * NMM)
                          nc.tensor.matmul(pha[:, ps], lhsT=w1a, rhs=sx[:, ms], start=True, stop=True)
                          nc.tensor.matmul(phb[:, ps], lhsT=w1b, rhs=sx[:, ms], start=True, stop=True)
                      # leaky(z) for chunk A on ScalarE (Prelu: z>0 ? z : alpha*z)
                      sha = hpool.tile([DA, BT], dt_mm, tag="sha")
                      nc.scalar.activation(sha[:], pha[:], A.Prelu,
                                           bias=bias_col(k, 0), scale=1.0, alpha=SLOPE)
                      # leaky(z) for chunk B on VectorE: t = 0.2*psum + 0.2*b1 ; max(psum + b1, t)
                      tb = hpool.tile([DB, BT], f32, tag="tb")
                      nc.vector.tensor_scalar(tb[:], phb[:], SLOPE, bias_col(k, 3, DB),
                                              OP.mult, OP.add)
                      shb = hpool.tile([DB, BT], dt_mm, tag="shb")
                      nc.vector.scalar_tensor_tensor(shb[:], phb[:], bias_col(k, 1, DB), tb[:],
                                                     OP.add, OP.max)
                      for i in range(n_mm):
                          ps = slice(i * NMM, (i + 1) * NMM)
                          nc.tensor.matmul(po[:, ps], lhsT=w2a, rhs=sha[:, ps], start=True, stop=False)
                          nc.tensor.matmul(po[:, ps], lhsT=w2b, rhs=shb[:, ps], start=False, stop=True)
                      nc.scalar.activation(so[:, hs], po[:], A.Prelu,
                                           bias=bias_col(k, 4), scale=1.0, alpha=SLOPE)
                  nc.sync.dma_start(oT[k * L : (k + 1) * L, :], so[:])
    nc.compile()
    return nc


def _prep(x, W1, b1, W2, b2, dt_name):
    """Host-side shard + layout prep."""
    _, np_dt = _DTYPES[dt_name]
    # diagonal W1 blocks: [k, d, m] -> lhsT [k, m, d]
    W1r = W1.reshape(K, DK, K, M)
    idx = np.arange(K)
    W1d = W1r[idx, :, idx, :]                       # [k, d, m]
    w1t = np.ascontiguousarray(W1d.transpose(0, 2, 1)).astype(np_dt)   # [k, m, d]
    w2t = np.ascontiguousarray(W2.transpose(0, 2, 1)).astype(np_dt)    # [k, d, l]
    bias = np.zeros((128, K, 6), np.float32)
    bias[:, :, 0] = b1[:, 0:DA].T
    bias[0:DB, :, 1] = b1[:, DA:DK].T
    bias[:, :, 2] = SLOPE * b1[:, 0:DA].T
    bias[0:DB, :, 3] = SLOPE * b1[:, DA:DK].T
    bias[:, :, 4] = b2.T
    bias[:, :, 5] = SLOPE * b2.T

    in_maps = []
    for c in range(NCORES):
        xT = np.ascontiguousarray(x[c * BL : (c + 1) * BL, :].T).astype(np_dt)
        in_maps.append({"xT": xT, "w1t": w1t, "w2t": w2t, "bias": bias})
    return in_maps



# ---------------- Variant C: fold 0.2*W2*z into a precomputed Wc matmul ----
# leaky(z) = 0.8*relu(z) + 0.2*z, so with r = relu(W1 x + b1):
#   o_pre = W2 leaky(z) + b2 = (0.2 W2 W1) x + (0.8 W2) r + (b2 + 0.2 W2 b1)
# The Wc = 0.2*W2*W1 matmul streams straight from x (no activation dep),
# r needs only a single relu op per element, and the output activation is a
# single 2048-wide Prelu per expert.

RA_ACT_FRAC = 0.5   # fraction of rA tiles on ScalarE (rest on VectorE)
NBT = 1024          # activation tile width
XBUFS = 2
OBUFS = 2
OUT_BF16 = True
PHBUFS = 1
POBUFS = 2
NMM = 512   # matmul moving free dim (fp32/fp32r hard cap 512)


def _build_c(dt_name, repeat=1):
    dt_mm, _ = _DTYPES[dt_name]
    f32 = mybir.dt.float32
    nc = bacc.Bacc("TRN2", target_bir_lowering=False, debug=False, num_devices=NCORES)

    xT = nc.dram_tensor("xT", [K * M, BL], dt_mm, kind="ExternalInput")
    w1t = nc.dram_tensor("w1t", [K, M, DK], dt_mm, kind="ExternalInput")    # [k][m, d]
    w2r = nc.dram_tensor("w2r", [K, DK, L], dt_mm, kind="ExternalInput")    # 0.8*W2, [k][d, l]
    wc = nc.dram_tensor("wc", [K, M, L], dt_mm, kind="ExternalInput")       # 0.2*(W2@W1d).T, [k][m, l]
    bias = nc.dram_tensor("bias", [128, K, 3], f32, kind="ExternalInput")
    o_dt = mybir.dt.bfloat16 if OUT_BF16 else f32
    oT = nc.dram_tensor("oT", [K * L, BL], o_dt, kind="ExternalOutput")

    nmm = NMM if dt_name == "bf16" else min(NMM, 512)   # fp32 moving-dim limit
    n_bt = BL // NBT            # activation tiles per expert
    n_mm = NBT // nmm           # matmuls per activation tile

    with TileContext(nc) as tc:
        with (
            tc.tile_pool(name="const", bufs=1) as cpool,
            tc.tile_pool(name="xin", bufs=XBUFS) as xpool,
            tc.tile_pool(name="h", bufs=2) as hpool,
            tc.tile_pool(name="o", bufs=OBUFS) as opool,
            tc.tile_pool(name="psum", bufs=1, space="PSUM") as psum,
        ):
            sw1 = cpool.tile([M, K, DK], dt_mm)
            sw2a = cpool.tile([DA, K, L], dt_mm)
            sw2b = cpool.tile([DB, K, L], dt_mm)
            swc = cpool.tile([M, K, L], dt_mm)
            sbias = cpool.tile([128, K, 3], f32)

            def bias_col(k, c, p=128):
                return sbias[0:p, k, c : c + 1]

            import contextlib
            loop_cm = tc.For_i(0, repeat, 1, hint_engines=(mybir.EngineType.PE,)) \
                if repeat > 1 else contextlib.nullcontext()
            with loop_cm:
              ract = 0
              for k in range(K):
                sx = xpool.tile([M, BL], dt_mm, tag="sx")
                nc.sync.dma_start(sx[:], xT[k * M : (k + 1) * M, :])
                if k == 0:
                    # bulk weight loads right after x(0) so expert 0 starts fast
                    nc.sync.dma_start(sw1[:], w1t.rearrange("k m d -> m k d"))
                    nc.sync.dma_start(swc[:], wc.rearrange("k m l -> m k l"))
                    nc.sync.dma_start(sw2a[:], w2r[:, 0:DA, :].rearrange("k d l -> d k l"))
                    nc.sync.dma_start(sw2b[:], w2r[:, DA:DK, :].rearrange("k d l -> d k l"))
                    nc.sync.dma_start(sbias[:], bias[:])
                so = opool.tile([L, BL], o_dt, tag="so")
                w1a = sw1[:, k, 0:DA]
                w1b = sw1[:, k, DA:DK]
                w2a = sw2a[:, k, :]
                w2b = sw2b[:, k, :]
                wck = swc[:, k, :]
                r_dt = f32 if dt_name == "fp32" else dt_mm
                ra = hpool.tile([DA, BL], r_dt, tag="ra")
                rb = hpool.tile([DB, BL], r_dt, tag="rb")
                po_tiles = {}

                def stage1(j):
                    nonlocal ract
                    pha = psum.tile([DA, NBT], f32, tag="pha", bufs=PHBUFS)
                    phb = psum.tile([DB, NBT], f32, tag="phb", bufs=PHBUFS)
                    for i in range(n_mm):
                        ms = slice(j * NBT + i * nmm, j * NBT + (i + 1) * nmm)
                        pp = slice(i * nmm, (i + 1) * nmm)
                        nc.tensor.matmul(pha[:, pp], lhsT=w1a, rhs=sx[:, ms], start=True, stop=True)
                        nc.tensor.matmul(phb[:, pp], lhsT=w1b, rhs=sx[:, ms], start=True, stop=True)
                    ps = slice(j * NBT, (j + 1) * NBT)
                    if (ract * 977) % 1000 < RA_ACT_FRAC * 1000:
                        nc.scalar.activation(ra[:, ps], pha[:], A.Relu,
                                             bias=bias_col(k, 0), scale=1.0)
                    else:
                        nc.vector.tensor_scalar(ra[:, ps], pha[:], bias_col(k, 0), 0.0,
                                                OP.add, OP.max)
                    ract += 1
                    nc.vector.tensor_scalar(rb[:, ps], phb[:], bias_col(k, 1, DB), 0.0,
                                            OP.add, OP.max)

                def stage2(j):
                    po = psum.tile([L, NBT], f32, tag="po", bufs=POBUFS, name=f"po_{k}_{j}")
                    for i in range(n_mm):
                        ms = slice(j * NBT + i * nmm, j * NBT + (i + 1) * nmm)
                        pp = slice(i * nmm, (i + 1) * nmm)
                        nc.tensor.matmul(po[:, pp], lhsT=wck, rhs=sx[:, ms], start=True, stop=False)
                        nc.tensor.matmul(po[:, pp], lhsT=w2a, rhs=ra[:, ms], start=False, stop=False)
                        nc.tensor.matmul(po[:, pp], lhsT=w2b, rhs=rb[:, ms], start=False, stop=True)
                    ps = slice(j * NBT, (j + 1) * NBT)
                    nc.scalar.activation(so[:, ps], po[:], A.Prelu,
                                         bias=bias_col(k, 2), scale=1.0, alpha=SLOPE)

                for j in range(n_bt + 1):
                    if j < n_bt:
                        stage1(j)
                    if j >= 1:
                        stage2(j - 1)
                nc.sync.dma_start(oT[k * L : (k + 1) * L, :], so[:])
    nc.compile()
    return nc


def _prep_c(x, W1, b1, W2, b2, dt_name):
    _, np_dt = _DTYPES[dt_name]
    W1r = W1.reshape(K, DK, K, M)
    idx = np.arange(K)
    W1d = W1r[idx, :, idx, :]                                            # [k, d, m]
    w1t = np.ascontiguousarray(W1d.transpose(0, 2, 1)).astype(np_dt)     # [k, m, d]
    w2r = np.ascontiguousarray((0.8 * W2).transpose(0, 2, 1)).astype(np_dt)
    wck = 0.2 * np.matmul(W2, W1d)                                       # [k, l, m]
    wc = np.ascontiguousarray(wck.transpose(0, 2, 1)).astype(np_dt)      # [k, m, l]
    b2p = b2 + 0.2 * np.einsum("kld,kd->kl", W2, b1)
    bias = np.zeros((128, K, 3), np.float32)
    bias[:, :, 0] = b1[:, 0:DA].T
    bias[0:DB, :, 1] = b1[:, DA:DK].T
    bias[:, :, 2] = b2p.T
    in_maps = []
    for c in range(NCORES):
        xTc = np.ascontiguousarray(x[c * BL : (c + 1) * BL, :].T).astype(np_dt)
        in_maps.append({"xT": xTc, "w1t": w1t, "w2r": w2r, "wc": wc, "bias": bias})
    return in_maps


# ---------------- Variants D/E: 512-wide tiles, streamed weights -----------
# E ("wc" math, default): po = wc.x + 0.8*W2a.relu(hA) + 0.8*W2b.relu(hB);
#   h acts are 1-pass relu on any engine, o act is Prelu.  PE floor
#   5 MM/tile = 68.3us/core.
# D ("4mm" math): po = W2a.leaky(hA) + W2b.leaky(hB); PE floor 54.6us but
#   leaky on DVE/Pool costs 2 passes -> act floor ~62us.
# Both: NBT=512 (1 PSUM bank/tile, all tags double-buffered), per-expert
# weight DMA (bufs=2) so startup and cross-iteration reloads overlap.

# Engine codes: "s" ScalarE 1-pass, "v" full on DVE (PSUM pass [+ SBUF max
# pass for leaky]).  GpSimd/Pool cannot read PSUM and rejects
# TensorScalarPtr outright, so acts run on ScalarE/DVE only.
NBT_D = 512           # tile width (1 PSUM bank per [*,512] f32 tile)
HA_PAT = ("v", "s", "v", "v")   # engine cycle for the hA act
HB_PAT = ("v", "v", "s", "v")   # engine cycle for the hB act
O_PAT = ("s",)                  # engine cycle for the output act
PH_BUFS = 2
PO_BUFS = 2
XBUFS_D = 3
WBUFS = 3


def _build_d(dt_name, repeat=1, four_mm=True):
    dt_mm, _ = _DTYPES[dt_name]
    f32 = mybir.dt.float32
    bf16 = mybir.dt.bfloat16
    nc = bacc.Bacc("TRN2", target_bir_lowering=False, debug=False, num_devices=NCORES)

    # packed per-expert weights: one DMA per expert.  Columns:
    #   [0:DK]            w1t[k]  ([m, d] lhsT)
    #   [DK:DK+L]         w2a     ([d(0:128), l] lhsT, 0.8x for wc variant)
    #   [DK+L:DK+2L]      w2b     ([d(128:240), l] on partitions 0:112)
    #   [DK+2L:DK+3L]     wc[k]   ([m, l] lhsT; only when four_mm=False)
    WCOLS = DK + (2 + (0 if four_mm else 1)) * L
    xT = nc.dram_tensor("xT", [K * M, BL], dt_mm, kind="ExternalInput")
    wpk = nc.dram_tensor("wpk", [K, M, WCOLS], dt_mm, kind="ExternalInput")
    bias = nc.dram_tensor("bias", [128, K, 3], f32, kind="ExternalInput")
    o_dt = bf16 if OUT_BF16 else f32
    oT = nc.dram_tensor("oT", [K * L, BL], o_dt, kind="ExternalOutput")

    n_bt = BL // NBT_D
    r_dt = bf16 if dt_name != "fp32" else f32

    with TileContext(nc) as tc:
        with (
            tc.tile_pool(name="const", bufs=1) as cpool,
            tc.tile_pool(name="wts", bufs=WBUFS) as wpool,
            tc.tile_pool(name="xin", bufs=XBUFS_D) as xpool,
            tc.tile_pool(name="h", bufs=2) as hpool,
            tc.tile_pool(name="u", bufs=3) as upool,
            tc.tile_pool(name="o", bufs=OBUFS) as opool,
            tc.tile_pool(name="psum", bufs=1, space="PSUM") as psum,
        ):
            sbias = cpool.tile([128, K, 3], f32)
            nc.sync.dma_start(sbias[:], bias[:])

            def bias_col(k, c, p=128):
                return sbias[0:p, k, c : c + 1]

            import contextlib
            loop_cm = tc.For_i(0, repeat, 1, hint_engines=(mybir.EngineType.PE,)) \
                if repeat > 1 else contextlib.nullcontext()
            with loop_cm:
              counts = {"hb": 0, "o": 0, "ha": 0}

              def act(eng, dst, src_psum, bcol, p, kind):
                  """dst = relu/leaky(src_psum + bias) via the chosen engine."""
                  if kind == "relu":
                      if eng == "s":
                          nc.scalar.activation(dst, src_psum, A.Relu,
                                               bias=bcol, scale=1.0)
                      else:
                          nc.vector.tensor_scalar(dst, src_psum, bcol, 0.0, OP.add, OP.max)
                  else:
                      if eng == "s":
                          nc.scalar.activation(dst, src_psum, A.Prelu,
                                               bias=bcol, scale=1.0, alpha=SLOPE)
                      else:
                          u = upool.tile([p, NBT_D], r_dt, tag="u" + eng)
                          nc.vector.tensor_scalar_add(u[:], src_psum, bcol)
                          nc.vector.scalar_tensor_tensor(dst, u[:], SLOPE, u[:], OP.mult, OP.max)

              h_kind = "leaky" if four_mm else "relu"

              # input prefetch runs PF experts ahead of the output DMA so the
              # (FIFO) HWDGE ring never gates sx/sw behind an oT that waits
              # on the last output act.
              fetched = {}

              def fetch(kk):
                  if kk >= K:
                      return
                  sx = xpool.tile([M, BL], dt_mm, tag="sx")
                  nc.sync.dma_start(sx[:], xT[kk * M : (kk + 1) * M, :])
                  sw = wpool.tile([M, WCOLS], dt_mm, tag="sw")
                  nc.sync.dma_start(sw[:], wpk[kk])
                  fetched[kk] = (sx, sw)

              PF = min(XBUFS_D, WBUFS) - 1
              for kk in range(PF):
                  fetch(kk)
              for k in range(K):
                  fetch(k + PF)
                  sx, sw = fetched.pop(k)

                  so = opool.tile([L, BL], o_dt, tag="so")
                  ra = hpool.tile([DA, BL], r_dt, tag="ra")
                  rb = hpool.tile([DB, BL], r_dt, tag="rb")
                  w1a = sw[:, 0:DA]
                  w1b = sw[:, DA:DK]
                  sw2a = sw[0:DA, DK:DK + L]
                  sw2b = sw[0:DB, DK + L:DK + 2 * L]
                  if not four_mm:
                      swc = sw[:, DK + 2 * L:DK + 3 * L]

                  def stage1(j):
                      ms = slice(j * NBT_D, (j + 1) * NBT_D)
                      pha = psum.tile([DA, NBT_D], f32, tag="pha", bufs=PH_BUFS)
                      phb = psum.tile([DB, NBT_D], f32, tag="phb", bufs=PH_BUFS)
                      nc.tensor.matmul(pha[:], lhsT=w1a, rhs=sx[:, ms], start=True, stop=True)
                      nc.tensor.matmul(phb[:], lhsT=w1b, rhs=sx[:, ms], start=True, stop=True)
                      ea = HA_PAT[counts["ha"] % len(HA_PAT)]; counts["ha"] += 1
                      act(ea, ra[:, ms], pha[:], bias_col(k, 0), DA, h_kind)
                      eb = HB_PAT[counts["hb"] % len(HB_PAT)]; counts["hb"] += 1
                      act(eb, rb[:, ms], phb[:], bias_col(k, 1, DB), DB, h_kind)

                  def stage2(j):
                      ms = slice(j * NBT_D, (j + 1) * NBT_D)
                      po = psum.tile([L, NBT_D], f32, tag="po", bufs=PO_BUFS)
                      if four_mm:
                          nc.tensor.matmul(po[:], lhsT=sw2a, rhs=ra[:, ms], start=True, stop=False)
                          nc.tensor.matmul(po[:], lhsT=sw2b, rhs=rb[:, ms], start=False, stop=True)
                      else:
                          nc.tensor.matmul(po[:], lhsT=swc, rhs=sx[:, ms], start=True, stop=False)
                          nc.tensor.matmul(po[:], lhsT=sw2a, rhs=ra[:, ms], start=False, stop=False)
                          nc.tensor.matmul(po[:], lhsT=sw2b, rhs=rb[:, ms], start=False, stop=True)
                      eo = O_PAT[counts["o"] % len(O_PAT)]; counts["o"] += 1
                      act(eo, so[:, ms], po[:], bias_col(k, 2), L, "leaky")

                  for j in range(n_bt + 1):
                      if j < n_bt:
                          stage1(j)
                      if j >= 1:
                          stage2(j - 1)
                  # last expert's output goes out on the ACT HWDGE ring so the
                  # next iteration's input prefetch (SP ring, FIFO) is not
                  # gated behind an oT that waits on the final output act.
                  oeng = nc.scalar if k == K - 1 else nc.sync
                  oeng.dma_start(oT[k * L : (k + 1) * L, :], so[:])
    nc.compile()
    return nc


def _build_e(dt_name, repeat=1):
    return _build_d(dt_name, repeat, four_mm=False)


def _prep_e(x, W1, b1, W2, b2, dt_name):
    """wc-variant weights (w1t, 0.8*W2, wc=0.2*W2@W1d, folded b2), packed."""
    _, np_dt = _DTYPES[dt_name]
    W1r = W1.reshape(K, DK, K, M)
    idx = np.arange(K)
    W1d = W1r[idx, :, idx, :]                                            # [k, d, m]
    w1t = W1d.transpose(0, 2, 1)                                         # [k, m, d]
    w2r = (0.8 * W2).transpose(0, 2, 1)                                  # [k, d, l]
    wck = 0.2 * np.matmul(W2, W1d)                                       # [k, l, m]
    wc = wck.transpose(0, 2, 1)                                          # [k, m, l]
    wpk = _pack_weights(w1t, w2r, wc, np_dt)
    b2p = b2 + 0.2 * np.einsum("kld,kd->kl", W2, b1)
    bias = np.zeros((128, K, 3), np.float32)
    bias[:, :, 0] = b1[:, 0:DA].T
    bias[0:DB, :, 1] = b1[:, DA:DK].T
    bias[:, :, 2] = b2p.T
    in_maps = []
    for c in range(NCORES):
        xTc = np.ascontiguousarray(x[c * BL : (c + 1) * BL, :].T).astype(np_dt)
        in_maps.append({"xT": xTc, "wpk": wpk, "bias": bias})
    return in_maps


def _pack_weights(w1t, w2t, wc, np_dt):
    """wpk[k] = [m/d, DK + 2L (+L)]: w1t | w2a | w2b | (wc)."""
    ncol = DK + 2 * L + (L if wc is not None else 0)
    wpk = np.zeros((K, M, ncol), np.float32)
    wpk[:, :, 0:DK] = w1t                       # [k, m, d]
    wpk[:, 0:DA, DK:DK + L] = w2t[:, 0:DA, :]   # [k, d(0:128), l]
    wpk[:, 0:DB, DK + L:DK + 2 * L] = w2t[:, DA:DK, :]
    if wc is not None:
        wpk[:, :, DK + 2 * L:DK + 3 * L] = wc
    return np.ascontiguousarray(wpk).astype(np_dt)


def _prep_d(x, W1, b1, W2, b2, dt_name):
    _, np_dt = _DTYPES[dt_name]
    W1r = W1.reshape(K, DK, K, M)
    idx = np.arange(K)
    W1d = W1r[idx, :, idx, :]                                            # [k, d, m]
    w1t = W1d.transpose(0, 2, 1)                                         # [k, m, d]
    w2t = W2.transpose(0, 2, 1)                                          # [k, d, l]
    wpk = _pack_weights(w1t, w2t, None, np_dt)
    bias = np.zeros((128, K, 3), np.float32)
    bias[:, :, 0] = b1[:, 0:DA].T
    bias[0:DB, :, 1] = b1[:, DA:DK].T
    bias[:, :, 2] = b2.T
    in_maps = []
    for c in range(NCORES):
        xTc = np.ascontiguousarray(x[c * BL : (c + 1) * BL, :].T).astype(np_dt)
        in_maps.append({"xT": xTc, "wpk": wpk, "bias": bias})
    return in_maps


_BUILDERS = {"a": (_build, _prep), "c": (_build_c, _prep_c),
             "d": (_build_d, _prep_d), "e": (_build_e, _prep_e)}

VARIANT = "e"   # "a" | "c" | "d" | "e"

def _run(x, W1, b1, W2, b2, repeat=1):
    x, W1, b1, W2, b2 = (np.asarray(a) for a in (x, W1, b1, W2, b2))
    key = (VARIANT, DT, repeat)
    if key not in _cache:
        _cache[key] = _BUILDERS[VARIANT][0](DT, repeat)
    nc = _cache[key]
    in_maps = _BUILDERS[VARIANT][1](x, W1, b1, W2, b2, DT)
    res = run_bass_kernel_spmd(nc, in_maps, list(range(NCORES)))
    out = np.empty((B, K * L), np.float32)
    for c in range(NCORES):
        out[c * BL : (c + 1) * BL, :] = res.results[c]["oT"].T.astype(np.float32)
    return out, res


def kernel(x, W1, b1, W2, b2):
    out, _ = _run(x, W1, b1, W2, b2)
    return out


def measure_hw_time(x, W1, b1, W2, b2, repeat=(10000, 20000), rounds=3):
    """Estimate per-pass on-device time: the kernel body runs inside a hardware
    For_i loop; per-pass time = slope of wall-clock between two large repeat
    counts (transfer/dispatch overheads cancel; axon tunnel noise ~0.1s forces
    large R)."""
    import time as _time
    in_maps = _BUILDERS[VARIANT][1](x, W1, b1, W2, b2, DT)
    r_lo, r_hi = repeat
    walls = {}
    for r in (r_lo, r_hi):
        key = (VARIANT, DT, r)
        if key not in _cache:
            _cache[key] = _BUILDERS[VARIANT][0](DT, r)
        nc = _cache[key]
        run_bass_kernel_spmd(nc, in_maps, list(range(NCORES)))  # warm (jit compile)
        best = float("inf")
        for _ in range(rounds):
            t0 = _time.perf_counter()
            run_bass_kernel_spmd(nc, in_maps, list(range(NCORES)))
            best = min(best, _time.perf_counter() - t0)
        walls[r] = best
    hw_s = (walls[r_hi] - walls[r_lo]) / (r_hi - r_lo)
    return hw_s * 1e9, walls



# revision 36
# speedup vs baseline: 1.2368x; 1.1203x over previous
"""Trainium2 Bass kernel for nn_CompositionalMLP_75763223101514.

Reference computation (per batch row b, expert k):
    xb = x.reshape(B, 16, 128)
    h  = leaky( einsum('bkm,kdm->bkd', xb, W1diag) + b1 )    # W1diag[k] = W1[k,:,k*128:(k+1)*128]
    o  = leaky( einsum('bkd,kld->bkl', h, W2) + b2 )
    out = o.reshape(B, 16*128)
with leaky(z) = z if z > 0 else 0.2 z.

Strategy: data-parallel over the batch dim across 8 NeuronCores (2048 rows
each), weights replicated.  On the host we pre-transpose each x shard to
feature-major [2048, 2048] so the contraction dim (m) lands on SBUF
partitions, extract the diagonal W1 blocks, and pre-transpose the weights
into lhsT layout.  Each core then runs, per expert k:

    MM1:  hT[d, b]  = sum_m W1T_k[m, d] * xT[k*128+m, b]     (PE, contraction 128)
    act:  h = leaky(hT + b1)  (ScalarE Prelu for the d<128 chunk + the output;
                               VectorE 2-op max(z, 0.2z) for the d>=128 chunk)
    MM2:  oT[l, b]  = sum_d W2T_k[d, l] * h[d, b]            (PE, contraction 240, accumulated)
    act:  o = leaky(oT + b2)  -> SBUF -> DMA to oT dram [k*128+l, b]

The host finally re-transposes each core's oT shard back to [2048, 2048]
batch-major and concatenates.

Matmul dtype: float32r (single-pass fp32 on the PE at full bf16 rate for
moving dim >= 256; measured max rel err ~1.4e-4 per matmul vs 2.5e-3 for
bf16).  Set DT = "bf16" to halve input DMA instead.
"""

import numpy as np
import ml_dtypes

import concourse.bacc as bacc
import concourse.mybir as mybir
from concourse.tile import TileContext
from concourse.bass_utils import run_bass_kernel_spmd

K, M, DK, L = 16, 128, 240, 128
B = 16384
NCORES = 8
BL = B // NCORES          # batch rows per core
SLOPE = 0.2
DA, DB = 128, DK - 128    # hidden split (PSUM partition limit)

DT = "bf16"               # "bf16" | "fp32r" | "fp32"
BT = 1024                 # activation tile width (columns of local batch)

_DTYPES = {
    "bf16": (mybir.dt.bfloat16, ml_dtypes.bfloat16),
    "fp32r": (mybir.dt.float32r, np.float32),
    "fp32": (mybir.dt.float32, np.float32),
}

A = mybir.ActivationFunctionType
OP = mybir.AluOpType

_cache = {}


def _build(dt_name, repeat=1):
    """One SPMD program; all cores run it on their own batch shard."""
    dt_mm, _ = _DTYPES[dt_name]
    f32 = mybir.dt.float32
    nc = bacc.Bacc("TRN2", target_bir_lowering=False, debug=False, num_devices=NCORES)

    xT = nc.dram_tensor("xT", [K * M, BL], dt_mm, kind="ExternalInput")
    w1t = nc.dram_tensor("w1t", [K, M, DK], dt_mm, kind="ExternalInput")   # [k][m, d]
    w2t = nc.dram_tensor("w2t", [K, DK, L], dt_mm, kind="ExternalInput")   # [k][d, l]
    # bias pack: [:, k, 0]=b1[:128]  [:112, k, 1]=b1[128:]  [:, k, 2]=0.2*b1[:128]
    #            [:112, k, 3]=0.2*b1[128:]  [:, k, 4]=b2  [:, k, 5]=0.2*b2
    bias = nc.dram_tensor("bias", [128, K, 6], f32, kind="ExternalInput")
    oT = nc.dram_tensor("oT", [K * L, BL], f32, kind="ExternalOutput")

    n_half = BL // BT           # halves per expert
    n_mm = BT // NMM            # matmuls per half per chunk

    with TileContext(nc) as tc:
        with (
            tc.tile_pool(name="const", bufs=1) as cpool,
            tc.tile_pool(name="xin", bufs=XBUFS) as xpool,
            tc.tile_pool(name="h", bufs=2) as hpool,
            tc.tile_pool(name="o", bufs=OBUFS) as opool,
            tc.tile_pool(name="psum", bufs=1, space="PSUM") as psum,
        ):
            # --- resident weights/biases ---
            sw1 = cpool.tile([M, K, DK], dt_mm)
            nc.sync.dma_start(sw1[:], w1t.rearrange("k m d -> m k d"))
            sw2a = cpool.tile([DA, K, L], dt_mm)
            nc.sync.dma_start(sw2a[:], w2t[:, 0:DA, :].rearrange("k d l -> d k l"))
            sw2b = cpool.tile([DB, K, L], dt_mm)
            nc.sync.dma_start(sw2b[:], w2t[:, DA:DK, :].rearrange("k d l -> d k l"))
            sbias = cpool.tile([128, K, 6], f32)
            nc.sync.dma_start(sbias[:], bias[:])

            def bias_col(k, c, p=128):
                return sbias[0:p, k, c : c + 1]

            import contextlib
            loop_cm = tc.For_i(0, repeat, 1, hint_engines=(mybir.EngineType.PE,)) \
                if repeat > 1 else contextlib.nullcontext()
            with loop_cm:
              for k in range(K):
                  sx = xpool.tile([M, BL], dt_mm, tag="sx")
                  nc.sync.dma_start(sx[:], xT[k * M : (k + 1) * M, :])
                  so = opool.tile([L, BL], o_dt, tag="so")
                  w1a = sw1[:, k, 0:DA]
                  w1b = sw1[:, k, DA:DK]
                  w2a = sw2a[:, k, :]
                  w2b = sw2b[:, k, :]
                  for h in range(n_half):
                      hs = slice(h * BT, (h + 1) * BT)
                      pha = psum.tile([DA, BT], f32, tag="pha", bufs=1)
                      phb = psum.tile([DB, BT], f32, tag="phb", bufs=1)
                      po = psum.tile([L, BT], f32, tag="po", bufs=2)
                      for i in range(n_mm):
                          ms = slice(h * BT + i * NMM, h * BT + (i + 1) * NMM)
                          ps = slice(i * NMM, (i + 1) * NMM)
                          nc.tensor.matmul(pha[:, ps], lhsT=w1a, rhs=sx[:, ms], start=True, stop=True)
                          nc.tensor.matmul(phb[:, ps], lhsT=w1b, rhs=sx[:, ms], start=True, stop=True)
                      # leaky(z) for chunk A on ScalarE (Prelu: z>0 ? z : alpha*z)
                      sha = hpool.tile([DA, BT], dt_mm, tag="sha")
                      nc.scalar.activation(sha[:], pha[:], A.Prelu,
                                           bias=bias_col(k, 0), scale=1.0, alpha=SLOPE)
                      # leaky(z) for chunk B on VectorE: t = 0.2*psum + 0.2*b1 ; max(psum + b1, t)
                      tb = hpool.tile([DB, BT], f32, tag="tb")
                      nc.vector.tensor_scalar(tb[:], phb[:], SLOPE, bias_col(k, 3, DB),
                                              OP.mult, OP.add)
                      shb = hpool.tile([DB, BT], dt_mm, tag="shb")
                      nc.vector.scalar_tensor_tensor(shb[:], phb[:], bias_col(k, 1, DB), tb[:],
                                                     OP.add, OP.max)
                      for i in range(n_mm):
                          ps = slice(i * NMM, (i + 1) * NMM)
                          nc.tensor.matmul(po[:, ps], lhsT=w2a, rhs=sha[:, ps], start=True, stop=False)
                          nc.tensor.matmul(po[:, ps], lhsT=w2b, rhs=shb[:, ps], start=False, stop=True)
                      nc.scalar.activation(so[:, hs], po[:], A.Prelu,
                                           bias=bias_col(k, 4), scale=1.0, alpha=SLOPE)
                  nc.sync.dma_start(oT[k * L : (k + 1) * L, :], so[:])
    nc.compile()
    return nc


def _prep(x, W1, b1, W2, b2, dt_name):
    """Host-side shard + layout prep."""
    _, np_dt = _DTYPES[dt_name]
    # diagonal W1 blocks: [k, d, m] -> lhsT [k, m, d]
    W1r = W1.reshape(K, DK, K, M)
    idx = np.arange(K)
    W1d = W1r[idx, :, idx, :]                       # [k, d, m]
    w1t = np.ascontiguousarray(W1d.transpose(0, 2, 1)).astype(np_dt)   # [k, m, d]
    w2t = np.ascontiguousarray(W2.transpose(0, 2, 1)).astype(np_dt)    # [k, d, l]
    bias = np.zeros((128, K, 6), np.float32)
    bias[:, :, 0] = b1[:, 0:DA].T
    bias[0:DB, :, 1] = b1[:, DA:DK].T
    bias[:, :, 2] = SLOPE * b1[:, 0:DA].T
    bias[0:DB, :, 3] = SLOPE * b1[:, DA:DK].T
    bias[:, :, 4] = b2.T
    bias[:, :, 5] = SLOPE * b2.T

    in_maps = []
    for c in range(NCORES):
        xT = np.ascontiguousarray(x[c * BL : (c + 1) * BL, :].T).astype(np_dt)
        in_maps.append({"xT": xT, "w1t": w1t, "w2t": w2t, "bias": bias})
    return in_maps



# ---------------- Variant C: fold 0.2*W2*z into a precomputed Wc matmul ----
# leaky(z) = 0.8*relu(z) + 0.2*z, so with r = relu(W1 x + b1):
#   o_pre = W2 leaky(z) + b2 = (0.2 W2 W1) x + (0.8 W2) r + (b2 + 0.2 W2 b1)
# The Wc = 0.2*W2*W1 matmul streams straight from x (no activation dep),
# r needs only a single relu op per element, and the output activation is a
# single 2048-wide Prelu per expert.

RA_ACT_FRAC = 0.5   # fraction of rA tiles on ScalarE (rest on VectorE)
NBT = 1024          # activation tile width
XBUFS = 2
OBUFS = 2
OUT_BF16 = True
PHBUFS = 1
POBUFS = 2
NMM = 512   # matmul moving free dim (fp32/fp32r hard cap 512)


def _build_c(dt_name, repeat=1):
    dt_mm, _ = _DTYPES[dt_name]
    f32 = mybir.dt.float32
    nc = bacc.Bacc("TRN2", target_bir_lowering=False, debug=False, num_devices=NCORES)

    xT = nc.dram_tensor("xT", [K * M, BL], dt_mm, kind="ExternalInput")
    w1t = nc.dram_tensor("w1t", [K, M, DK], dt_mm, kind="ExternalInput")    # [k][m, d]
    w2r = nc.dram_tensor("w2r", [K, DK, L], dt_mm, kind="ExternalInput")    # 0.8*W2, [k][d, l]
    wc = nc.dram_tensor("wc", [K, M, L], dt_mm, kind="ExternalInput")       # 0.2*(W2@W1d).T, [k][m, l]
    bias = nc.dram_tensor("bias", [128, K, 3], f32, kind="ExternalInput")
    o_dt = mybir.dt.bfloat16 if OUT_BF16 else f32
    oT = nc.dram_tensor("oT", [K * L, BL], o_dt, kind="ExternalOutput")

    nmm = NMM if dt_name == "bf16" else min(NMM, 512)   # fp32 moving-dim limit
    n_bt = BL // NBT            # activation tiles per expert
    n_mm = NBT // nmm           # matmuls per activation tile

    with TileContext(nc) as tc:
        with (
            tc.tile_pool(name="const", bufs=1) as cpool,
            tc.tile_pool(name="xin", bufs=XBUFS) as xpool,
            tc.tile_pool(name="h", bufs=2) as hpool,
            tc.tile_pool(name="o", bufs=OBUFS) as opool,
            tc.tile_pool(name="psum", bufs=1, space="PSUM") as psum,
        ):
            sw1 = cpool.tile([M, K, DK], dt_mm)
            sw2a = cpool.tile([DA, K, L], dt_mm)
            sw2b = cpool.tile([DB, K, L], dt_mm)
            swc = cpool.tile([M, K, L], dt_mm)
            sbias = cpool.tile([128, K, 3], f32)

            def bias_col(k, c, p=128):
                return sbias[0:p, k, c : c + 1]

            import contextlib
            loop_cm = tc.For_i(0, repeat, 1, hint_engines=(mybir.EngineType.PE,)) \
                if repeat > 1 else contextlib.nullcontext()
            with loop_cm:
              ract = 0
              for k in range(K):
                sx = xpool.tile([M, BL], dt_mm, tag="sx")
                nc.sync.dma_start(sx[:], xT[k * M : (k + 1) * M, :])
                if k == 0:
                    # bulk weight loads right after x(0) so expert 0 starts fast
                    nc.sync.dma_start(sw1[:], w1t.rearrange("k m d -> m k d"))
                    nc.sync.dma_start(swc[:], wc.rearrange("k m l -> m k l"))
                    nc.sync.dma_start(sw2a[:], w2r[:, 0:DA, :].rearrange("k d l -> d k l"))
                    nc.sync.dma_start(sw2b[:], w2r[:, DA:DK, :].rearrange("k d l -> d k l"))
                    nc.sync.dma_start(sbias[:], bias[:])
                so = opool.tile([L, BL], o_dt, tag="so")
                w1a = sw1[:, k, 0:DA]
                w1b = sw1[:, k, DA:DK]
                w2a = sw2a[:, k, :]
                w2b = sw2b[:, k, :]
                wck = swc[:, k, :]
                r_dt = f32 if dt_name == "fp32" else dt_mm
                ra = hpool.tile([DA, BL], r_dt, tag="ra")
                rb = hpool.tile([DB, BL], r_dt, tag="rb")
                po_tiles = {}

                def stage1(j):
                    nonlocal ract
                    pha = psum.tile([DA, NBT], f32, tag="pha", bufs=PHBUFS)
                    phb = psum.tile([DB, NBT], f32, tag="phb", bufs=PHBUFS)
                    for i in range(n_mm):
                        ms = slice(j * NBT + i * nmm, j * NBT + (i + 1) * nmm)
                        pp = slice(i * nmm, (i + 1) * nmm)
                        nc.tensor.matmul(pha[:, pp], lhsT=w1a, rhs=sx[:, ms], start=True, stop=True)
                        nc.tensor.matmul(phb[:, pp], lhsT=w1b, rhs=sx[:, ms], start=True, stop=True)
                    ps = slice(j * NBT, (j + 1) * NBT)
                    if (ract * 977) % 1000 < RA_ACT_FRAC * 1000:
                        nc.scalar.activation(ra[:, ps], pha[:], A.Relu,
                                             bias=bias_col(k, 0), scale=1.0)
                    else:
                        nc.vector.tensor_scalar(ra[:, ps], pha[:], bias_col(k, 0), 0.0,
                                                OP.add, OP.max)
                    ract += 1
                    nc.vector.tensor_scalar(rb[:, ps], phb[:], bias_col(k, 1, DB), 0.0,
                                            OP.add, OP.max)

                def stage2(j):
                    po = psum.tile([L, NBT], f32, tag="po", bufs=POBUFS, name=f"po_{k}_{j}")
                    for i in range(n_mm):
                        ms = slice(j * NBT + i * nmm, j * NBT + (i + 1) * nmm)
                        pp = slice(i * nmm, (i + 1) * nmm)
                        nc.tensor.matmul(po[:, pp], lhsT=wck, rhs=sx[:, ms], start=True, stop=False)
                        nc.tensor.matmul(po[:, pp], lhsT=w2a, rhs=ra[:, ms], start=False, stop=False)
                        nc.tensor.matmul(po[:, pp], lhsT=w2b, rhs=rb[:, ms], start=False, stop=True)
                    ps = slice(j * NBT, (j + 1) * NBT)
                    nc.scalar.activation(so[:, ps], po[:], A.Prelu,
                                         bias=bias_col(k, 2), scale=1.0, alpha=SLOPE)

                for j in range(n_bt + 1):
                    if j < n_bt:
                        stage1(j)
                    if j >= 1:
                        stage2(j - 1)
                nc.sync.dma_start(oT[k * L : (k + 1) * L, :], so[:])
    nc.compile()
    return nc


def _prep_c(x, W1, b1, W2, b2, dt_name):
    _, np_dt = _DTYPES[dt_name]
    W1r = W1.reshape(K, DK, K, M)
    idx = np.arange(K)
    W1d = W1r[idx, :, idx, :]                                            # [k, d, m]
    w1t = np.ascontiguousarray(W1d.transpose(0, 2, 1)).astype(np_dt)     # [k, m, d]
    w2r = np.ascontiguousarray((0.8 * W2).transpose(0, 2, 1)).astype(np_dt)
    wck = 0.2 * np.matmul(W2, W1d)                                       # [k, l, m]
    wc = np.ascontiguousarray(wck.transpose(0, 2, 1)).astype(np_dt)      # [k, m, l]
    b2p = b2 + 0.2 * np.einsum("kld,kd->kl", W2, b1)
    bias = np.zeros((128, K, 3), np.float32)
    bias[:, :, 0] = b1[:, 0:DA].T
    bias[0:DB, :, 1] = b1[:, DA:DK].T
    bias[:, :, 2] = b2p.T
    in_maps = []
    for c in range(NCORES):
        xTc = np.ascontiguousarray(x[c * BL : (c + 1) * BL, :].T).astype(np_dt)
        in_maps.append({"xT": xTc, "w1t": w1t, "w2r": w2r, "wc": wc, "bias": bias})
    return in_maps


# ---------------- Variants D/E: 512-wide tiles, streamed weights -----------
# E ("wc" math, default): po = wc.x + 0.8*W2a.relu(hA) + 0.8*W2b.relu(hB);
#   h acts are 1-pass relu on any engine, o act is Prelu.  PE floor
#   5 MM/tile = 68.3us/core.
# D ("4mm" math): po = W2a.leaky(hA) + W2b.leaky(hB); PE floor 54.6us but
#   leaky on DVE/Pool costs 2 passes -> act floor ~62us.
# Both: NBT=512 (1 PSUM bank/tile, all tags double-buffered), per-expert
# weight DMA (bufs=2) so startup and cross-iteration reloads overlap.

# Engine codes: "s" ScalarE 1-pass, "v" full on DVE (PSUM pass [+ SBUF max
# pass for leaky]).  GpSimd/Pool cannot read PSUM and rejects
# TensorScalarPtr outright, so acts run on ScalarE/DVE only.
NBT_D = 512           # tile width (1 PSUM bank per [*,512] f32 tile)
HA_PAT = ("v", "s", "v", "v")   # engine cycle for the hA act
HB_PAT = ("v", "v", "s", "v")   # engine cycle for the hB act
O_PAT = ("s",)                  # engine cycle for the output act
PH_BUFS = 2
PO_BUFS = 2
XBUFS_D = 3
WBUFS = 3


def _build_d(dt_name, repeat=1, four_mm=True):
    dt_mm, _ = _DTYPES[dt_name]
    f32 = mybir.dt.float32
    bf16 = mybir.dt.bfloat16
    nc = bacc.Bacc("TRN2", target_bir_lowering=False, debug=False, num_devices=NCORES)

    # packed per-expert weights: one DMA per expert.  Columns:
    #   [0:DKv]           w1t[k]  ([m, d] lhsT)
    #   [DKv:DKv+L]       w2a     ([d(0:128), l] lhsT, 0.8x for wc variant)
    #   [DKv+L:DKv+2L]    w2b     ([d(128:DKv), l] on partitions 0:DBv)
    #   [DKv+2L:DKv+3L]   wc[k]   ([m, l] lhsT; only when four_mm=False)
    # four_mm pads the hidden dim 240->256 (zero weight cols / rows / bias)
    # so every stationary operand is a full 128 columns (FWL-eligible).
    DKv = 256 if four_mm else DK
    DBv = DKv - DA
    WCOLS = DKv + (2 + (0 if four_mm else 1)) * L
    xT = nc.dram_tensor("xT", [K * M, BL], dt_mm, kind="ExternalInput")
    wpk = nc.dram_tensor("wpk", [K, M, WCOLS], dt_mm, kind="ExternalInput")
    bias = nc.dram_tensor("bias", [128, K, 3], f32, kind="ExternalInput")
    o_dt = bf16 if OUT_BF16 else f32
    oT = nc.dram_tensor("oT", [K * L, BL], o_dt, kind="ExternalOutput")

    n_bt = BL // NBT_D
    r_dt = bf16 if dt_name != "fp32" else f32

    with TileContext(nc) as tc:
        with (
            tc.tile_pool(name="const", bufs=1) as cpool,
            tc.tile_pool(name="wts", bufs=WBUFS) as wpool,
            tc.tile_pool(name="xin", bufs=XBUFS_D) as xpool,
            tc.tile_pool(name="h", bufs=2) as hpool,
            tc.tile_pool(name="u", bufs=3) as upool,
            tc.tile_pool(name="o", bufs=OBUFS) as opool,
            tc.tile_pool(name="psum", bufs=1, space="PSUM") as psum,
        ):
            sbias = cpool.tile([128, K, 3], f32)
            nc.sync.dma_start(sbias[:], bias[:])

            def bias_col(k, c, p=128):
                return sbias[0:p, k, c : c + 1]

            import contextlib
            loop_cm = tc.For_i(0, repeat, 1, hint_engines=(mybir.EngineType.PE,)) \
                if repeat > 1 else contextlib.nullcontext()
            with loop_cm:
              counts = {"hb": 0, "o": 0, "ha": 0}

              def act(eng, dst, src_psum, bcol, p, kind):
                  """dst = relu/leaky(src_psum + bias) via the chosen engine."""
                  if kind == "relu":
                      if eng == "s":
                          nc.scalar.activation(dst, src_psum, A.Relu,
                                               bias=bcol, scale=1.0)
                      else:
                          nc.vector.tensor_scalar(dst, src_psum, bcol, 0.0, OP.add, OP.max)
                  else:
                      if eng == "s":
                          nc.scalar.activation(dst, src_psum, A.Prelu,
                                               bias=bcol, scale=1.0, alpha=SLOPE)
                      else:
                          u = upool.tile([p, NBT_D], r_dt, tag="u" + eng)
                          nc.vector.tensor_scalar_add(u[:], src_psum, bcol)
                          nc.vector.scalar_tensor_tensor(dst, u[:], SLOPE, u[:], OP.mult, OP.max)

              h_kind = "leaky" if four_mm else "relu"
              # leaky costs DVE ~1.05us/tile vs ScalarE 0.6 -> shift the mix
              # toward ScalarE for the 4mm (leaky-h) variant.
              ha_pat = ("s", "v") if four_mm else HA_PAT
              hb_pat = ("v", "s") if four_mm else HB_PAT

              # input prefetch runs PF experts ahead of the output DMA so the
              # (FIFO) HWDGE ring never gates sx/sw behind an oT that waits
              # on the last output act.
              fetched = {}

              def fetch(kk):
                  if kk >= K:
                      return
                  sx = xpool.tile([M, BL], dt_mm, tag="sx")
                  nc.sync.dma_start(sx[:], xT[kk * M : (kk + 1) * M, :])
                  sw = wpool.tile([M, WCOLS], dt_mm, tag="sw")
                  nc.sync.dma_start(sw[:], wpk[kk])
                  fetched[kk] = (sx, sw)

              PF = min(XBUFS_D, WBUFS) - 1
              for kk in range(PF):
                  fetch(kk)
              for k in range(K):
                  fetch(k + PF)
                  sx, sw = fetched.pop(k)

                  so = opool.tile([L, BL], o_dt, tag="so")
                  ra = hpool.tile([DA, BL], r_dt, tag="ra")
                  rb = hpool.tile([DBv, BL], r_dt, tag="rb")
                  w1a = sw[:, 0:DA]
                  w1b = sw[:, DA:DKv]
                  sw2a = sw[0:DA, DKv:DKv + L]
                  sw2b = sw[0:DBv, DKv + L:DKv + 2 * L]
                  if not four_mm:
                      swc = sw[:, DKv + 2 * L:DKv + 3 * L]

                  def stage1(j):
                      ms = slice(j * NBT_D, (j + 1) * NBT_D)
                      pha = psum.tile([DA, NBT_D], f32, tag="pha", bufs=PH_BUFS)
                      phb = psum.tile([DBv, NBT_D], f32, tag="phb", bufs=PH_BUFS)
                      nc.tensor.matmul(pha[:], lhsT=w1a, rhs=sx[:, ms], start=True, stop=True)
                      nc.tensor.matmul(phb[:], lhsT=w1b, rhs=sx[:, ms], start=True, stop=True)
                      ea = ha_pat[counts["ha"] % len(ha_pat)]; counts["ha"] += 1
                      act(ea, ra[:, ms], pha[:], bias_col(k, 0), DA, h_kind)
                      eb = hb_pat[counts["hb"] % len(hb_pat)]; counts["hb"] += 1
                      act(eb, rb[:, ms], phb[:], bias_col(k, 1, DBv), DBv, h_kind)

                  def stage2(j):
                      ms = slice(j * NBT_D, (j + 1) * NBT_D)
                      po = psum.tile([L, NBT_D], f32, tag="po", bufs=PO_BUFS)
                      if four_mm:
                          nc.tensor.matmul(po[:], lhsT=sw2a, rhs=ra[:, ms], start=True, stop=False)
                          nc.tensor.matmul(po[:], lhsT=sw2b, rhs=rb[:, ms], start=False, stop=True)
                      else:
                          nc.tensor.matmul(po[:], lhsT=swc, rhs=sx[:, ms], start=True, stop=False)
                          nc.tensor.matmul(po[:], lhsT=sw2a, rhs=ra[:, ms], start=False, stop=False)
                          nc.tensor.matmul(po[:], lhsT=sw2b, rhs=rb[:, ms], start=False, stop=True)
                      eo = O_PAT[counts["o"] % len(O_PAT)]; counts["o"] += 1
                      act(eo, so[:, ms], po[:], bias_col(k, 2), L, "leaky")

                  for j in range(n_bt + 1):
                      if j < n_bt:
                          stage1(j)
                      if j >= 1:
                          stage2(j - 1)
                  # last expert's output goes out on the ACT HWDGE ring so the
                  # next iteration's input prefetch (SP ring, FIFO) is not
                  # gated behind an oT that waits on the final output act.
                  oeng = nc.scalar if k == K - 1 else nc.sync
                  oeng.dma_start(oT[k * L : (k + 1) * L, :], so[:])
    nc.compile()
    return nc


def _build_e(dt_name, repeat=1):
    return _build_d(dt_name, repeat, four_mm=False)


def _prep_e(x, W1, b1, W2, b2, dt_name):
    """wc-variant weights (w1t, 0.8*W2, wc=0.2*W2@W1d, folded b2), packed."""
    _, np_dt = _DTYPES[dt_name]
    W1r = W1.reshape(K, DK, K, M)
    idx = np.arange(K)
    W1d = W1r[idx, :, idx, :]                                            # [k, d, m]
    w1t = W1d.transpose(0, 2, 1)                                         # [k, m, d]
    w2r = (0.8 * W2).transpose(0, 2, 1)                                  # [k, d, l]
    wck = 0.2 * np.matmul(W2, W1d)                                       # [k, l, m]
    wc = wck.transpose(0, 2, 1)                                          # [k, m, l]
    wpk = _pack_weights(w1t, w2r, wc, np_dt)
    b2p = b2 + 0.2 * np.einsum("kld,kd->kl", W2, b1)
    bias = np.zeros((128, K, 3), np.float32)
    bias[:, :, 0] = b1[:, 0:DA].T
    bias[0:DB, :, 1] = b1[:, DA:DK].T
    bias[:, :, 2] = b2p.T
    in_maps = []
    for c in range(NCORES):
        xTc = np.ascontiguousarray(x[c * BL : (c + 1) * BL, :].T).astype(np_dt)
        in_maps.append({"xT": xTc, "wpk": wpk, "bias": bias})
    return in_maps


def _pack_weights(w1t, w2t, wc, np_dt, dkv=DK):
    """wpk[k] = [m/d, DKv + 2L (+L)]: w1t | w2a | w2b | (wc); DKv>DK pads
    the hidden dim with zero weight columns/rows (output unchanged)."""
    ncol = dkv + 2 * L + (L if wc is not None else 0)
    wpk = np.zeros((K, M, ncol), np.float32)
    wpk[:, :, 0:DK] = w1t                       # [k, m, d]
    wpk[:, 0:DA, dkv:dkv + L] = w2t[:, 0:DA, :]   # [k, d(0:128), l]
    wpk[:, 0:DK - DA, dkv + L:dkv + 2 * L] = w2t[:, DA:DK, :]
    if wc is not None:
        wpk[:, :, dkv + 2 * L:dkv + 3 * L] = wc
    return np.ascontiguousarray(wpk).astype(np_dt)


def _prep_d(x, W1, b1, W2, b2, dt_name):
    _, np_dt = _DTYPES[dt_name]
    W1r = W1.reshape(K, DK, K, M)
    idx = np.arange(K)
    W1d = W1r[idx, :, idx, :]                                            # [k, d, m]
    w1t = W1d.transpose(0, 2, 1)                                         # [k, m, d]
    w2t = W2.transpose(0, 2, 1)                                          # [k, d, l]
    wpk = _pack_weights(w1t, w2t, None, np_dt, dkv=256)
    bias = np.zeros((128, K, 3), np.float32)
    bias[:, :, 0] = b1[:, 0:DA].T
    bias[0:DB, :, 1] = b1[:, DA:DK].T
    bias[:, :, 2] = b2.T
    in_maps = []
    for c in range(NCORES):
        xTc = np.ascontiguousarray(x[c * BL : (c + 1) * BL, :].T).astype(np_dt)
        in_maps.append({"xT": xTc, "wpk": wpk, "bias": bias})
    return in_maps


_BUILDERS = {"a": (_build, _prep), "c": (_build_c, _prep_c),
             "d": (_build_d, _prep_d), "e": (_build_e, _prep_e)}

VARIANT = "d"   # "a" | "c" | "d" | "e"

def _run(x, W1, b1, W2, b2, repeat=1):
    x, W1, b1, W2, b2 = (np.asarray(a) for a in (x, W1, b1, W2, b2))
    key = (VARIANT, DT, repeat)
    if key not in _cache:
        _cache[key] = _BUILDERS[VARIANT][0](DT, repeat)
    nc = _cache[key]
    in_maps = _BUILDERS[VARIANT][1](x, W1, b1, W2, b2, DT)
    res = run_bass_kernel_spmd(nc, in_maps, list(range(NCORES)))
    out = np.empty((B, K * L), np.float32)
    for c in range(NCORES):
        out[c * BL : (c + 1) * BL, :] = res.results[c]["oT"].T.astype(np.float32)
    return out, res


def kernel(x, W1, b1, W2, b2):
    out, _ = _run(x, W1, b1, W2, b2)
    return out


def measure_hw_time(x, W1, b1, W2, b2, repeat=(10000, 20000), rounds=3):
    """Estimate per-pass on-device time: the kernel body runs inside a hardware
    For_i loop; per-pass time = slope of wall-clock between two large repeat
    counts (transfer/dispatch overheads cancel; axon tunnel noise ~0.1s forces
    large R)."""
    import time as _time
    in_maps = _BUILDERS[VARIANT][1](x, W1, b1, W2, b2, DT)
    r_lo, r_hi = repeat
    walls = {}
    for r in (r_lo, r_hi):
        key = (VARIANT, DT, r)
        if key not in _cache:
            _cache[key] = _BUILDERS[VARIANT][0](DT, r)
        nc = _cache[key]
        run_bass_kernel_spmd(nc, in_maps, list(range(NCORES)))  # warm (jit compile)
        best = float("inf")
        for _ in range(rounds):
            t0 = _time.perf_counter()
            run_bass_kernel_spmd(nc, in_maps, list(range(NCORES)))
            best = min(best, _time.perf_counter() - t0)
        walls[r] = best
    hw_s = (walls[r_hi] - walls[r_lo]) / (r_hi - r_lo)
    return hw_s * 1e9, walls



# revision 37
# speedup vs baseline: 1.2515x; 1.0119x over previous
"""Trainium2 Bass kernel for nn_CompositionalMLP_75763223101514.

Reference computation (per batch row b, expert k):
    xb = x.reshape(B, 16, 128)
    h  = leaky( einsum('bkm,kdm->bkd', xb, W1diag) + b1 )    # W1diag[k] = W1[k,:,k*128:(k+1)*128]
    o  = leaky( einsum('bkd,kld->bkl', h, W2) + b2 )
    out = o.reshape(B, 16*128)
with leaky(z) = z if z > 0 else 0.2 z.

Strategy: data-parallel over the batch dim across 8 NeuronCores (2048 rows
each), weights replicated.  On the host we pre-transpose each x shard to
feature-major [2048, 2048] so the contraction dim (m) lands on SBUF
partitions, extract the diagonal W1 blocks, and pre-transpose the weights
into lhsT layout.  Each core then runs, per expert k:

    MM1:  hT[d, b]  = sum_m W1T_k[m, d] * xT[k*128+m, b]     (PE, contraction 128)
    act:  h = leaky(hT + b1)  (ScalarE Prelu for the d<128 chunk + the output;
                               VectorE 2-op max(z, 0.2z) for the d>=128 chunk)
    MM2:  oT[l, b]  = sum_d W2T_k[d, l] * h[d, b]            (PE, contraction 240, accumulated)
    act:  o = leaky(oT + b2)  -> SBUF -> DMA to oT dram [k*128+l, b]

The host finally re-transposes each core's oT shard back to [2048, 2048]
batch-major and concatenates.

Matmul dtype: float32r (single-pass fp32 on the PE at full bf16 rate for
moving dim >= 256; measured max rel err ~1.4e-4 per matmul vs 2.5e-3 for
bf16).  Set DT = "bf16" to halve input DMA instead.
"""

import numpy as np
import ml_dtypes

import concourse.bacc as bacc
import concourse.mybir as mybir
from concourse.tile import TileContext
from concourse.bass_utils import run_bass_kernel_spmd

K, M, DK, L = 16, 128, 240, 128
B = 16384
NCORES = 8
BL = B // NCORES          # batch rows per core
SLOPE = 0.2
DA, DB = 128, DK - 128    # hidden split (PSUM partition limit)

DT = "bf16"               # "bf16" | "fp32r" | "fp32"
BT = 1024                 # activation tile width (columns of local batch)

_DTYPES = {
    "bf16": (mybir.dt.bfloat16, ml_dtypes.bfloat16),
    "fp32r": (mybir.dt.float32r, np.float32),
    "fp32": (mybir.dt.float32, np.float32),
}

A = mybir.ActivationFunctionType
OP = mybir.AluOpType

_cache = {}


def _build(dt_name, repeat=1):
    """One SPMD program; all cores run it on their own batch shard."""
    dt_mm, _ = _DTYPES[dt_name]
    f32 = mybir.dt.float32
    nc = bacc.Bacc("TRN2", target_bir_lowering=False, debug=False, num_devices=NCORES)

    xT = nc.dram_tensor("xT", [K * M, BL], dt_mm, kind="ExternalInput")
    w1t = nc.dram_tensor("w1t", [K, M, DK], dt_mm, kind="ExternalInput")   # [k][m, d]
    w2t = nc.dram_tensor("w2t", [K, DK, L], dt_mm, kind="ExternalInput")   # [k][d, l]
    # bias pack: [:, k, 0]=b1[:128]  [:112, k, 1]=b1[128:]  [:, k, 2]=0.2*b1[:128]
    #            [:112, k, 3]=0.2*b1[128:]  [:, k, 4]=b2  [:, k, 5]=0.2*b2
    bias = nc.dram_tensor("bias", [128, K, 6], f32, kind="ExternalInput")
    oT = nc.dram_tensor("oT", [K * L, BL], f32, kind="ExternalOutput")

    n_half = BL // BT           # halves per expert
    n_mm = BT // NMM            # matmuls per half per chunk

    with TileContext(nc) as tc:
        with (
            tc.tile_pool(name="const", bufs=1) as cpool,
            tc.tile_pool(name="xin", bufs=XBUFS) as xpool,
            tc.tile_pool(name="h", bufs=2) as hpool,
            tc.tile_pool(name="o", bufs=OBUFS) as opool,
            tc.tile_pool(name="psum", bufs=1, space="PSUM") as psum,
        ):
            # --- resident weights/biases ---
            sw1 = cpool.tile([M, K, DK], dt_mm)
            nc.sync.dma_start(sw1[:], w1t.rearrange("k m d -> m k d"))
            sw2a = cpool.tile([DA, K, L], dt_mm)
            nc.sync.dma_start(sw2a[:], w2t[:, 0:DA, :].rearrange("k d l -> d k l"))
            sw2b = cpool.tile([DB, K, L], dt_mm)
            nc.sync.dma_start(sw2b[:], w2t[:, DA:DK, :].rearrange("k d l -> d k l"))
            sbias = cpool.tile([128, K, 6], f32)
            nc.sync.dma_start(sbias[:], bias[:])

            def bias_col(k, c, p=128):
                return sbias[0:p, k, c : c + 1]

            import contextlib
            loop_cm = tc.For_i(0, repeat, 1, hint_engines=(mybir.EngineType.PE,)) \
                if repeat > 1 else contextlib.nullcontext()
            with loop_cm:
              for k in range(K):
                  sx = xpool.tile([M, BL], dt_mm, tag="sx")
                  nc.sync.dma_start(sx[:], xT[k * M : (k + 1) * M, :])
                  so = opool.tile([L, BL], o_dt, tag="so")
                  w1a = sw1[:, k, 0:DA]
                  w1b = sw1[:, k, DA:DK]
                  w2a = sw2a[:, k, :]
                  w2b = sw2b[:, k, :]
                  for h in range(n_half):
                      hs = slice(h * BT, (h + 1) * BT)
                      pha = psum.tile([DA, BT], f32, tag="pha", bufs=1)
                      phb = psum.tile([DB, BT], f32, tag="phb", bufs=1)
                      po = psum.tile([L, BT], f32, tag="po", bufs=2)
                      for i in range(n_mm):
                          ms = slice(h * BT + i * NMM, h * BT + (i + 1) * NMM)
                          ps = slice(i * NMM, (i + 1) * NMM)
                          nc.tensor.matmul(pha[:, ps], lhsT=w1a, rhs=sx[:, ms], start=True, stop=True)
                          nc.tensor.matmul(phb[:, ps], lhsT=w1b, rhs=sx[:, ms], start=True, stop=True)
                      # leaky(z) for chunk A on ScalarE (Prelu: z>0 ? z : alpha*z)
                      sha = hpool.tile([DA, BT], dt_mm, tag="sha")
                      nc.scalar.activation(sha[:], pha[:], A.Prelu,
                                           bias=bias_col(k, 0), scale=1.0, alpha=SLOPE)
                      # leaky(z) for chunk B on VectorE: t = 0.2*psum + 0.2*b1 ; max(psum + b1, t)
                      tb = hpool.tile([DB, BT], f32, tag="tb")
                      nc.vector.tensor_scalar(tb[:], phb[:], SLOPE, bias_col(k, 3, DB),
                                              OP.mult, OP.add)
                      shb = hpool.tile([DB, BT], dt_mm, tag="shb")
                      nc.vector.scalar_tensor_tensor(shb[:], phb[:], bias_col(k, 1, DB), tb[:],
                                                     OP.add, OP.max)
                      for i in range(n_mm):
                          ps = slice(i * NMM, (i + 1) * NMM)
                          nc.tensor.matmul(po[:, ps], lhsT=w2a, rhs=sha[:, ps], start=True, stop=False)
                          nc.tensor.matmul(po[:, ps], lhsT=w2b, rhs=shb[:, ps], start=False, stop=True)
                      nc.scalar.activation(so[:, hs], po[:], A.Prelu,
                                           bias=bias_col(k, 4), scale=1.0, alpha=SLOPE)
                  nc.sync.dma_start(oT[k * L : (k + 1) * L, :], so[:])
    nc.compile()
    return nc


def _prep(x, W1, b1, W2, b2, dt_name):
    """Host-side shard + layout prep."""
    _, np_dt = _DTYPES[dt_name]
    # diagonal W1 blocks: [k, d, m] -> lhsT [k, m, d]
    W1r = W1.reshape(K, DK, K, M)
    idx = np.arange(K)
    W1d = W1r[idx, :, idx, :]                       # [k, d, m]
    w1t = np.ascontiguousarray(W1d.transpose(0, 2, 1)).astype(np_dt)   # [k, m, d]
    w2t = np.ascontiguousarray(W2.transpose(0, 2, 1)).astype(np_dt)    # [k, d, l]
    bias = np.zeros((128, K, 6), np.float32)
    bias[:, :, 0] = b1[:, 0:DA].T
    bias[0:DB, :, 1] = b1[:, DA:DK].T
    bias[:, :, 2] = SLOPE * b1[:, 0:DA].T
    bias[0:DB, :, 3] = SLOPE * b1[:, DA:DK].T
    bias[:, :, 4] = b2.T
    bias[:, :, 5] = SLOPE * b2.T

    in_maps = []
    for c in range(NCORES):
        xT = np.ascontiguousarray(x[c * BL : (c + 1) * BL, :].T).astype(np_dt)
        in_maps.append({"xT": xT, "w1t": w1t, "w2t": w2t, "bias": bias})
    return in_maps



# ---------------- Variant C: fold 0.2*W2*z into a precomputed Wc matmul ----
# leaky(z) = 0.8*relu(z) + 0.2*z, so with r = relu(W1 x + b1):
#   o_pre = W2 leaky(z) + b2 = (0.2 W2 W1) x + (0.8 W2) r + (b2 + 0.2 W2 b1)
# The Wc = 0.2*W2*W1 matmul streams straight from x (no activation dep),
# r needs only a single relu op per element, and the output activation is a
# single 2048-wide Prelu per expert.

RA_ACT_FRAC = 0.5   # fraction of rA tiles on ScalarE (rest on VectorE)
NBT = 1024          # activation tile width
XBUFS = 2
OBUFS = 2
OUT_BF16 = True
PHBUFS = 1
POBUFS = 2
NMM = 512   # matmul moving free dim (fp32/fp32r hard cap 512)


def _build_c(dt_name, repeat=1):
    dt_mm, _ = _DTYPES[dt_name]
    f32 = mybir.dt.float32
    nc = bacc.Bacc("TRN2", target_bir_lowering=False, debug=False, num_devices=NCORES)

    xT = nc.dram_tensor("xT", [K * M, BL], dt_mm, kind="ExternalInput")
    w1t = nc.dram_tensor("w1t", [K, M, DK], dt_mm, kind="ExternalInput")    # [k][m, d]
    w2r = nc.dram_tensor("w2r", [K, DK, L], dt_mm, kind="ExternalInput")    # 0.8*W2, [k][d, l]
    wc = nc.dram_tensor("wc", [K, M, L], dt_mm, kind="ExternalInput")       # 0.2*(W2@W1d).T, [k][m, l]
    bias = nc.dram_tensor("bias", [128, K, 3], f32, kind="ExternalInput")
    o_dt = mybir.dt.bfloat16 if OUT_BF16 else f32
    oT = nc.dram_tensor("oT", [K * L, BL], o_dt, kind="ExternalOutput")

    nmm = NMM if dt_name == "bf16" else min(NMM, 512)   # fp32 moving-dim limit
    n_bt = BL // NBT            # activation tiles per expert
    n_mm = NBT // nmm           # matmuls per activation tile

    with TileContext(nc) as tc:
        with (
            tc.tile_pool(name="const", bufs=1) as cpool,
            tc.tile_pool(name="xin", bufs=XBUFS) as xpool,
            tc.tile_pool(name="h", bufs=2) as hpool,
            tc.tile_pool(name="o", bufs=OBUFS) as opool,
            tc.tile_pool(name="psum", bufs=1, space="PSUM") as psum,
        ):
            sw1 = cpool.tile([M, K, DK], dt_mm)
            sw2a = cpool.tile([DA, K, L], dt_mm)
            sw2b = cpool.tile([DB, K, L], dt_mm)
            swc = cpool.tile([M, K, L], dt_mm)
            sbias = cpool.tile([128, K, 3], f32)

            def bias_col(k, c, p=128):
                return sbias[0:p, k, c : c + 1]

            import contextlib
            loop_cm = tc.For_i(0, repeat, 1, hint_engines=(mybir.EngineType.PE,)) \
                if repeat > 1 else contextlib.nullcontext()
            with loop_cm:
              ract = 0
              for k in range(K):
                sx = xpool.tile([M, BL], dt_mm, tag="sx")
                nc.sync.dma_start(sx[:], xT[k * M : (k + 1) * M, :])
                if k == 0:
                    # bulk weight loads right after x(0) so expert 0 starts fast
                    nc.sync.dma_start(sw1[:], w1t.rearrange("k m d -> m k d"))
                    nc.sync.dma_start(swc[:], wc.rearrange("k m l -> m k l"))
                    nc.sync.dma_start(sw2a[:], w2r[:, 0:DA, :].rearrange("k d l -> d k l"))
                    nc.sync.dma_start(sw2b[:], w2r[:, DA:DK, :].rearrange("k d l -> d k l"))
                    nc.sync.dma_start(sbias[:], bias[:])
                so = opool.tile([L, BL], o_dt, tag="so")
                w1a = sw1[:, k, 0:DA]
                w1b = sw1[:, k, DA:DK]
                w2a = sw2a[:, k, :]
                w2b = sw2b[:, k, :]
                wck = swc[:, k, :]
                r_dt = f32 if dt_name == "fp32" else dt_mm
                ra = hpool.tile([DA, BL], r_dt, tag="ra")
                rb = hpool.tile([DB, BL], r_dt, tag="rb")
                po_tiles = {}

                def stage1(j):
                    nonlocal ract
                    pha = psum.tile([DA, NBT], f32, tag="pha", bufs=PHBUFS)
                    phb = psum.tile([DB, NBT], f32, tag="phb", bufs=PHBUFS)
                    for i in range(n_mm):
                        ms = slice(j * NBT + i * nmm, j * NBT + (i + 1) * nmm)
                        pp = slice(i * nmm, (i + 1) * nmm)
                        nc.tensor.matmul(pha[:, pp], lhsT=w1a, rhs=sx[:, ms], start=True, stop=True)
                        nc.tensor.matmul(phb[:, pp], lhsT=w1b, rhs=sx[:, ms], start=True, stop=True)
                    ps = slice(j * NBT, (j + 1) * NBT)
                    if (ract * 977) % 1000 < RA_ACT_FRAC * 1000:
                        nc.scalar.activation(ra[:, ps], pha[:], A.Relu,
                                             bias=bias_col(k, 0), scale=1.0)
                    else:
                        nc.vector.tensor_scalar(ra[:, ps], pha[:], bias_col(k, 0), 0.0,
                                                OP.add, OP.max)
                    ract += 1
                    nc.vector.tensor_scalar(rb[:, ps], phb[:], bias_col(k, 1, DB), 0.0,
                                            OP.add, OP.max)

                def stage2(j):
                    po = psum.tile([L, NBT], f32, tag="po", bufs=POBUFS, name=f"po_{k}_{j}")
                    for i in range(n_mm):
                        ms = slice(j * NBT + i * nmm, j * NBT + (i + 1) * nmm)
                        pp = slice(i * nmm, (i + 1) * nmm)
                        nc.tensor.matmul(po[:, pp], lhsT=wck, rhs=sx[:, ms], start=True, stop=False)
                        nc.tensor.matmul(po[:, pp], lhsT=w2a, rhs=ra[:, ms], start=False, stop=False)
                        nc.tensor.matmul(po[:, pp], lhsT=w2b, rhs=rb[:, ms], start=False, stop=True)
                    ps = slice(j * NBT, (j + 1) * NBT)
                    nc.scalar.activation(so[:, ps], po[:], A.Prelu,
                                         bias=bias_col(k, 2), scale=1.0, alpha=SLOPE)

                for j in range(n_bt + 1):
                    if j < n_bt:
                        stage1(j)
                    if j >= 1:
                        stage2(j - 1)
                nc.sync.dma_start(oT[k * L : (k + 1) * L, :], so[:])
    nc.compile()
    return nc


def _prep_c(x, W1, b1, W2, b2, dt_name):
    _, np_dt = _DTYPES[dt_name]
    W1r = W1.reshape(K, DK, K, M)
    idx = np.arange(K)
    W1d = W1r[idx, :, idx, :]                                            # [k, d, m]
    w1t = np.ascontiguousarray(W1d.transpose(0, 2, 1)).astype(np_dt)     # [k, m, d]
    w2r = np.ascontiguousarray((0.8 * W2).transpose(0, 2, 1)).astype(np_dt)
    wck = 0.2 * np.matmul(W2, W1d)                                       # [k, l, m]
    wc = np.ascontiguousarray(wck.transpose(0, 2, 1)).astype(np_dt)      # [k, m, l]
    b2p = b2 + 0.2 * np.einsum("kld,kd->kl", W2, b1)
    bias = np.zeros((128, K, 3), np.float32)
    bias[:, :, 0] = b1[:, 0:DA].T
    bias[0:DB, :, 1] = b1[:, DA:DK].T
    bias[:, :, 2] = b2p.T
    in_maps = []
    for c in range(NCORES):
        xTc = np.ascontiguousarray(x[c * BL : (c + 1) * BL, :].T).astype(np_dt)
        in_maps.append({"xT": xTc, "w1t": w1t, "w2r": w2r, "wc": wc, "bias": bias})
    return in_maps


# ---------------- Variants D/E: 512-wide tiles, streamed weights -----------
# E ("wc" math, default): po = wc.x + 0.8*W2a.relu(hA) + 0.8*W2b.relu(hB);
#   h acts are 1-pass relu on any engine, o act is Prelu.  PE floor
#   5 MM/tile = 68.3us/core.
# D ("4mm" math): po = W2a.leaky(hA) + W2b.leaky(hB); PE floor 54.6us but
#   leaky on DVE/Pool costs 2 passes -> act floor ~62us.
# Both: NBT=512 (1 PSUM bank/tile, all tags double-buffered), per-expert
# weight DMA (bufs=2) so startup and cross-iteration reloads overlap.

# Engine codes: "s" ScalarE 1-pass, "v" full on DVE (PSUM pass [+ SBUF max
# pass for leaky]).  GpSimd/Pool cannot read PSUM and rejects
# TensorScalarPtr outright, so acts run on ScalarE/DVE only.
NBT_D = 512           # tile width (1 PSUM bank per [*,512] f32 tile)
HA_PAT = ("v", "s", "v", "v")   # engine cycle for the hA act
HB_PAT = ("v", "v", "s", "v")   # engine cycle for the hB act
O_PAT = ("s",)                  # engine cycle for the output act
PH_BUFS = 3
PO_BUFS = 2
XBUFS_D = 4
WBUFS = 4


def _build_d(dt_name, repeat=1, four_mm=True):
    dt_mm, _ = _DTYPES[dt_name]
    f32 = mybir.dt.float32
    bf16 = mybir.dt.bfloat16
    nc = bacc.Bacc("TRN2", target_bir_lowering=False, debug=False, num_devices=NCORES)

    # packed per-expert weights: one DMA per expert.  Columns:
    #   [0:DKv]           w1t[k]  ([m, d] lhsT)
    #   [DKv:DKv+L]       w2a     ([d(0:128), l] lhsT, 0.8x for wc variant)
    #   [DKv+L:DKv+2L]    w2b     ([d(128:DKv), l] on partitions 0:DBv)
    #   [DKv+2L:DKv+3L]   wc[k]   ([m, l] lhsT; only when four_mm=False)
    # four_mm pads the hidden dim 240->256 (zero weight cols / rows / bias)
    # so every stationary operand is a full 128 columns (FWL-eligible).
    DKv = 256 if four_mm else DK
    DBv = DKv - DA
    WCOLS = DKv + (2 + (0 if four_mm else 1)) * L
    xT = nc.dram_tensor("xT", [K * M, BL], dt_mm, kind="ExternalInput")
    wpk = nc.dram_tensor("wpk", [K, M, WCOLS], dt_mm, kind="ExternalInput")
    bias = nc.dram_tensor("bias", [128, K, 3], f32, kind="ExternalInput")
    o_dt = bf16 if OUT_BF16 else f32
    oT = nc.dram_tensor("oT", [K * L, BL], o_dt, kind="ExternalOutput")

    n_bt = BL // NBT_D
    r_dt = bf16 if dt_name != "fp32" else f32

    with TileContext(nc) as tc:
        with (
            tc.tile_pool(name="const", bufs=1) as cpool,
            tc.tile_pool(name="wts", bufs=WBUFS) as wpool,
            tc.tile_pool(name="xin", bufs=XBUFS_D) as xpool,
            tc.tile_pool(name="h", bufs=2) as hpool,
            tc.tile_pool(name="u", bufs=3) as upool,
            tc.tile_pool(name="o", bufs=OBUFS) as opool,
            tc.tile_pool(name="psum", bufs=1, space="PSUM") as psum,
        ):
            sbias = cpool.tile([128, K, 3], f32)
            nc.sync.dma_start(sbias[:], bias[:])

            def bias_col(k, c, p=128):
                return sbias[0:p, k, c : c + 1]

            import contextlib
            loop_cm = tc.For_i(0, repeat, 1, hint_engines=(mybir.EngineType.PE,)) \
                if repeat > 1 else contextlib.nullcontext()
            with loop_cm:
              counts = {"hb": 0, "o": 0, "ha": 0}

              def act(eng, dst, src_psum, bcol, p, kind):
                  """dst = relu/leaky(src_psum + bias) via the chosen engine."""
                  if kind == "relu":
                      if eng == "s":
                          nc.scalar.activation(dst, src_psum, A.Relu,
                                               bias=bcol, scale=1.0)
                      else:
                          nc.vector.tensor_scalar(dst, src_psum, bcol, 0.0, OP.add, OP.max)
                  else:
                      if eng == "s":
                          nc.scalar.activation(dst, src_psum, A.Prelu,
                                               bias=bcol, scale=1.0, alpha=SLOPE)
                      else:
                          u = upool.tile([p, NBT_D], r_dt, tag="u" + eng)
                          nc.vector.tensor_scalar_add(u[:], src_psum, bcol)
                          nc.vector.scalar_tensor_tensor(dst, u[:], SLOPE, u[:], OP.mult, OP.max)

              h_kind = "leaky" if four_mm else "relu"
              # leaky costs DVE ~1.05us/tile vs ScalarE 0.6 -> shift the mix
              # toward ScalarE for the 4mm (leaky-h) variant.
              ha_pat = ("s", "v") if four_mm else HA_PAT
              hb_pat = ("v", "s") if four_mm else HB_PAT

              # input prefetch runs PF experts ahead of the output DMA so the
              # (FIFO) HWDGE ring never gates sx/sw behind an oT that waits
              # on the last output act.
              fetched = {}

              def fetch(kk):
                  if kk >= K:
                      return
                  sx = xpool.tile([M, BL], dt_mm, tag="sx")
                  nc.sync.dma_start(sx[:], xT[kk * M : (kk + 1) * M, :])
                  sw = wpool.tile([M, WCOLS], dt_mm, tag="sw")
                  nc.sync.dma_start(sw[:], wpk[kk])
                  fetched[kk] = (sx, sw)

              PF = min(XBUFS_D, WBUFS) - 1
              for kk in range(PF):
                  fetch(kk)
              for k in range(K):
                  fetch(k + PF)
                  sx, sw = fetched.pop(k)

                  so = opool.tile([L, BL], o_dt, tag="so")
                  ra = hpool.tile([DA, BL], r_dt, tag="ra")
                  rb = hpool.tile([DBv, BL], r_dt, tag="rb")
                  w1a = sw[:, 0:DA]
                  w1b = sw[:, DA:DKv]
                  sw2a = sw[0:DA, DKv:DKv + L]
                  sw2b = sw[0:DBv, DKv + L:DKv + 2 * L]
                  if not four_mm:
                      swc = sw[:, DKv + 2 * L:DKv + 3 * L]

                  def stage1(j):
                      ms = slice(j * NBT_D, (j + 1) * NBT_D)
                      pha = psum.tile([DA, NBT_D], f32, tag="pha", bufs=PH_BUFS)
                      phb = psum.tile([DBv, NBT_D], f32, tag="phb", bufs=PH_BUFS)
                      nc.tensor.matmul(pha[:], lhsT=w1a, rhs=sx[:, ms], start=True, stop=True)
                      nc.tensor.matmul(phb[:], lhsT=w1b, rhs=sx[:, ms], start=True, stop=True)
                      ea = ha_pat[counts["ha"] % len(ha_pat)]; counts["ha"] += 1
                      act(ea, ra[:, ms], pha[:], bias_col(k, 0), DA, h_kind)
                      eb = hb_pat[counts["hb"] % len(hb_pat)]; counts["hb"] += 1
                      act(eb, rb[:, ms], phb[:], bias_col(k, 1, DBv), DBv, h_kind)

                  def stage2(j):
                      ms = slice(j * NBT_D, (j + 1) * NBT_D)
                      po = psum.tile([L, NBT_D], f32, tag="po", bufs=PO_BUFS)
                      if four_mm:
                          nc.tensor.matmul(po[:], lhsT=sw2a, rhs=ra[:, ms], start=True, stop=False)
                          nc.tensor.matmul(po[:], lhsT=sw2b, rhs=rb[:, ms], start=False, stop=True)
                      else:
                          nc.tensor.matmul(po[:], lhsT=swc, rhs=sx[:, ms], start=True, stop=False)
                          nc.tensor.matmul(po[:], lhsT=sw2a, rhs=ra[:, ms], start=False, stop=False)
                          nc.tensor.matmul(po[:], lhsT=sw2b, rhs=rb[:, ms], start=False, stop=True)
                      eo = O_PAT[counts["o"] % len(O_PAT)]; counts["o"] += 1
                      act(eo, so[:, ms], po[:], bias_col(k, 2), L, "leaky")

                  for j in range(n_bt + 1):
                      if j < n_bt:
                          stage1(j)
                      if j >= 1:
                          stage2(j - 1)
                  # last expert's output goes out on the ACT HWDGE ring so the
                  # next iteration's input prefetch (SP ring, FIFO) is not
                  # gated behind an oT that waits on the final output act.
                  oeng = nc.scalar if k == K - 1 else nc.sync
                  oeng.dma_start(oT[k * L : (k + 1) * L, :], so[:])
    nc.compile()
    return nc


def _build_e(dt_name, repeat=1):
    return _build_d(dt_name, repeat, four_mm=False)


def _prep_e(x, W1, b1, W2, b2, dt_name):
    """wc-variant weights (w1t, 0.8*W2, wc=0.2*W2@W1d, folded b2), packed."""
    _, np_dt = _DTYPES[dt_name]
    W1r = W1.reshape(K, DK, K, M)
    idx = np.arange(K)
    W1d = W1r[idx, :, idx, :]                                            # [k, d, m]
    w1t = W1d.transpose(0, 2, 1)                                         # [k, m, d]
    w2r = (0.8 * W2).transpose(0, 2, 1)                                  # [k, d, l]
    wck = 0.2 * np.matmul(W2, W1d)                                       # [k, l, m]
    wc = wck.transpose(0, 2, 1)                                          # [k, m, l]
    wpk = _pack_weights(w1t, w2r, wc, np_dt)
    b2p = b2 + 0.2 * np.einsum("kld,kd->kl", W2, b1)
    bias = np.zeros((128, K, 3), np.float32)
    bias[:, :, 0] = b1[:, 0:DA].T
    bias[0:DB, :, 1] = b1[:, DA:DK].T
    bias[:, :, 2] = b2p.T
    in_maps = []
    for c in range(NCORES):
        xTc = np.ascontiguousarray(x[c * BL : (c + 1) * BL, :].T).astype(np_dt)
        in_maps.append({"xT": xTc, "wpk": wpk, "bias": bias})
    return in_maps


def _pack_weights(w1t, w2t, wc, np_dt, dkv=DK):
    """wpk[k] = [m/d, DKv + 2L (+L)]: w1t | w2a | w2b | (wc); DKv>DK pads
    the hidden dim with zero weight columns/rows (output unchanged)."""
    ncol = dkv + 2 * L + (L if wc is not None else 0)
    wpk = np.zeros((K, M, ncol), np.float32)
    wpk[:, :, 0:DK] = w1t                       # [k, m, d]
    wpk[:, 0:DA, dkv:dkv + L] = w2t[:, 0:DA, :]   # [k, d(0:128), l]
    wpk[:, 0:DK - DA, dkv + L:dkv + 2 * L] = w2t[:, DA:DK, :]
    if wc is not None:
        wpk[:, :, dkv + 2 * L:dkv + 3 * L] = wc
    return np.ascontiguousarray(wpk).astype(np_dt)


def _prep_d(x, W1, b1, W2, b2, dt_name):
    _, np_dt = _DTYPES[dt_name]
    W1r = W1.reshape(K, DK, K, M)
    idx = np.arange(K)
    W1d = W1r[idx, :, idx, :]                                            # [k, d, m]
    w1t = W1d.transpose(0, 2, 1)                                         # [k, m, d]
    w2t = W2.transpose(0, 2, 1)                                          # [k, d, l]
    wpk = _pack_weights(w1t, w2t, None, np_dt, dkv=256)
    bias = np.zeros((128, K, 3), np.float32)
    bias[:, :, 0] = b1[:, 0:DA].T
    bias[0:DB, :, 1] = b1[:, DA:DK].T
    bias[:, :, 2] = b2.T
    in_maps = []
    for c in range(NCORES):
        xTc = np.ascontiguousarray(x[c * BL : (c + 1) * BL, :].T).astype(np_dt)
        in_maps.append({"xT": xTc, "wpk": wpk, "bias": bias})
    return in_maps


_BUILDERS = {"a": (_build, _prep), "c": (_build_c, _prep_c),
             "d": (_build_d, _prep_d), "e": (_build_e, _prep_e)}

VARIANT = "d"   # "a" | "c" | "d" | "e"

def _run(x, W1, b1, W2, b2, repeat=1):
    x, W1, b1, W2, b2 = (np.asarray(a) for a in (x, W1, b1, W2, b2))
    key = (VARIANT, DT, repeat)
    if key not in _cache:
        _cache[key] = _BUILDERS[VARIANT][0](DT, repeat)
    nc = _cache[key]
    in_maps = _BUILDERS[VARIANT][1](x, W1, b1, W2, b2, DT)
    res = run_bass_kernel_spmd(nc, in_maps, list(range(NCORES)))
    out = np.empty((B, K * L), np.float32)
    for c in range(NCORES):
        out[c * BL : (c + 1) * BL, :] = res.results[c]["oT"].T.astype(np.float32)
    return out, res


def kernel(x, W1, b1, W2, b2):
    out, _ = _run(x, W1, b1, W2, b2)
    return out


def measure_hw_time(x, W1, b1, W2, b2, repeat=(10000, 20000), rounds=3):
    """Estimate per-pass on-device time: the kernel body runs inside a hardware
    For_i loop; per-pass time = slope of wall-clock between two large repeat
    counts (transfer/dispatch overheads cancel; axon tunnel noise ~0.1s forces
    large R)."""
    import time as _time
    in_maps = _BUILDERS[VARIANT][1](x, W1, b1, W2, b2, DT)
    r_lo, r_hi = repeat
    walls = {}
    for r in (r_lo, r_hi):
        key = (VARIANT, DT, r)
        if key not in _cache:
            _cache[key] = _BUILDERS[VARIANT][0](DT, r)
        nc = _cache[key]
        run_bass_kernel_spmd(nc, in_maps, list(range(NCORES)))  # warm (jit compile)
        best = float("inf")
        for _ in range(rounds):
            t0 = _time.perf_counter()
            run_bass_kernel_spmd(nc, in_maps, list(range(NCORES)))
            best = min(best, _time.perf_counter() - t0)
        walls[r] = best
    hw_s = (walls[r_hi] - walls[r_lo]) / (r_hi - r_lo)
    return hw_s * 1e9, walls

